# revision 1
# baseline (speedup 1.0000x reference)
"""Trainium2 Bass kernel for nn_LocalGeoAgg (gnn_message_passing).

Strategy: data-parallel over batch B=8 across the 8 NeuronCores (one
sample per core). All convs are 1x1 so everything is per-point except
the training-mode BatchNorm statistics (and the global std of rel0),
which are all-reduced across cores (sync-BN) with 5 small AllReduces.

Layout: channels on partitions, points (G*K = 65536) on the free dim.
Activations are fp16 (x-path SBUF-resident; 64-channel h-path staged in
DRAM as packed [128, 32768] halves); matmuls run fp16 with f32 PSUM
accumulation; statistics are computed in f32 from pre-rounding PSUM.

Conv biases bd/bu are dropped: training-mode BN subtracts the batch
mean, which cancels any per-channel additive constant exactly.
"""

import sys

sys.path.insert(0, "/opt/trn_rl_repo")

import contextlib

import numpy as np

from concourse import bacc, bass, mybir, tile
from concourse import bass_utils, masks

dt = mybir.dt
AF = mybir.ActivationFunctionType
ALU = mybir.AluOpType
AX = mybir.AxisListType

B, G, KNN = 8, 2048, 32
P = G * KNN            # 65536 points per core
NP = 512               # points per tile
NT = P // NP           # 128 tiles
NJ = NT // 2           # 64 coupled (A, B) tile pairs
HALF = P // 2          # 32768
EPS = 1e-5
N_GLOBAL = B * P       # BN normalization count
N3 = B * P * 3         # rel0 element count (std)

_CACHE = {}


def _build(n_cores=8, use_cc=True):
    nc = bacc.Bacc("TRN2", target_bir_lowering=False, debug=False,
                   num_devices=n_cores)

    f32, f16 = dt.float32, dt.float16

    # ---- per-core external inputs -------------------------------------
    knn_feat = nc.dram_tensor("knn_feat", [P, 67], f32, kind="ExternalInput").ap()
    knn_xyz = nc.dram_tensor("knn_xyz", [128, 1536], f32, kind="ExternalInput").ap()
    lc_small = nc.dram_tensor("lc_small", [128, 48], f32, kind="ExternalInput").ap()
    lc_feat = nc.dram_tensor("lc_feat", [G, 64], f32, kind="ExternalInput").ap()
    w1aT = nc.dram_tensor("w1aT", [67, 128], f16, kind="ExternalInput").ap()
    w1bT = nc.dram_tensor("w1bT", [64, 128], f16, kind="ExternalInput").ap()
    wdT = nc.dram_tensor("wdT", [2, 128, 64], f16, kind="ExternalInput").ap()
    wuT = nc.dram_tensor("wuT", [2, 64, 128], f16, kind="ExternalInput").ap()
    gam = nc.dram_tensor("gam", [5, 128], f32, kind="ExternalInput").ap()
    bet = nc.dram_tensor("bet", [5, 128], f32, kind="ExternalInput").ap()
    out = nc.dram_tensor("out", [128, P], f32, kind="ExternalOutput").ap()

    rg = [list(range(n_cores))]

    with tile.TileContext(nc) as tc:
        with contextlib.ExitStack() as stack:
            pers = stack.enter_context(tc.tile_pool(name="pers", bufs=1))
            dram = stack.enter_context(tc.tile_pool(name="dram", bufs=1, space="DRAM"))

            # persistent SBUF residents
            x_slot = pers.tile([128, P], f16, name="x_slot")
            lcT = pers.tile([64, G], f16, name="lcT")
            st = pers.tile([128, NT, 6], f32, name="st")

            # DRAM-staged packed h tensors (ping-pong)
            h_dram = dram.tile([128, HALF], f16, name="h_dram")   # h0 / h1
            t_dram = dram.tile([128, HALF], f16, name="t_dram")   # t / t1

            # small weights / params
            w1a_s = pers.tile([67, 128], f16, name="w1a_s")
            w1b_s = pers.tile([64, 128], f16, name="w1b_s")
            wd_s = [pers.tile([128, 64], f16, name=f"wd_s{i}") for i in range(2)]
            wu_s = [pers.tile([128, 128], f16, name=f"wu_s{i}") for i in range(2)]
            ones1 = pers.tile([1, 128], f16, name="ones1")
            ident = pers.tile([128, 128], f16, name="ident")
            nc.sync.dma_start(w1a_s[:], w1aT[:])
            nc.sync.dma_start(w1b_s[:], w1bT[:])
            for i in range(2):
                nc.sync.dma_start(wd_s[i][:], wdT[i])
                # up weights: rows 0-63 AND rows 64-127 (row tiling pair)
                nc.sync.dma_start(wu_s[i][0:64, :], wuT[i])
                nc.sync.dma_start(wu_s[i][64:128, :], wuT[i])
            nc.vector.memset(ones1[:], 1.0)
            masks.make_identity(nc, ident[:])

            a_p = [pers.tile([128, 1], f32, name=f"a_p{i}") for i in range(5)]
            b_p = [pers.tile([128, 1], f32, name=f"b_p{i}") for i in range(5)]
            c_eps = pers.tile([128, 1], f32, name="c_eps")
            nc.vector.memset(c_eps[:], EPS)
            gam_s = pers.tile([128, 5], f32, name="gam_s")
            bet_s = pers.tile([128, 5], f32, name="bet_s")
            nc.sync.dma_start(gam_s[:], gam[:].rearrange("l c -> c l"))
            nc.sync.dma_start(bet_s[:], bet[:].rearrange("l c -> c l"))

            w_row = dram.tile([P], f16, name="w_row")

            def do_allreduce(idx):
                if use_cc:
                    nc.gpsimd.collective_compute(
                        "AllReduce", ALU.add, ins=[pay_i[idx].opt()],
                        outs=[pay_o[idx].opt()], replica_groups=rg)
                else:
                    nc.sync.dma_start(pay_o[idx][:], pay_i[idx][:])
            pay_i = [dram.tile([512], f32, name=f"pay_i{i}") for i in range(5)]
            pay_o = [dram.tile([512], f32, name=f"pay_o{i}") for i in range(5)]

            # ---------- helpers ------------------------------------------
            def stats_to_sums(ag, n, npart):
                """[npart,2] (mean,var) -> (sum, sumsq)."""
                i = stats_to_sums.i = stats_to_sums.i + 1
                sums = pers.tile([128, 2], f32, name=f"sums{i}")
                m2 = pers.tile([128, 1], f32, name=f"m2_{i}")
                nc.vector.tensor_tensor(m2[:npart], ag[:npart, 0:1], ag[:npart, 0:1], ALU.mult)
                nc.scalar.mul(sums[:npart, 0:1], ag[:npart, 0:1], float(n))
                nc.vector.tensor_tensor(sums[:npart, 1:2], ag[:npart, 1:2], m2[:npart], ALU.add)
                nc.scalar.mul(sums[:npart, 1:2], sums[:npart, 1:2], float(n))
                return sums

            stats_to_sums.i = 0

            def affine_from_sums(back, li, npart, n_total):
                """back [npart,2] global (sum,sumsq) -> a_p[li], b_p[li]."""
                mean = pers.tile([128, 1], f32, name=f"mean{li}")
                var = pers.tile([128, 1], f32, name=f"var{li}")
                m2 = pers.tile([128, 1], f32, name=f"m2g{li}")
                sig = pers.tile([128, 1], f32, name=f"sig{li}")
                nc.scalar.mul(mean[:npart], back[:npart, 0:1], 1.0 / n_total)
                nc.vector.tensor_tensor(m2[:npart], mean[:npart], mean[:npart], ALU.mult)
                nc.vector.scalar_tensor_tensor(
                    var[:npart], back[:npart, 1:2], 1.0 / n_total, m2[:npart],
                    ALU.mult, ALU.subtract)
                nc.scalar.activation(sig[:npart], var[:npart], AF.Sqrt, bias=c_eps[:npart])
                nc.vector.reciprocal(sig[:npart], sig[:npart])
                nc.vector.tensor_tensor(a_p[li][:npart], gam_s[:npart, li:li + 1],
                                        sig[:npart], ALU.mult)
                nc.vector.tensor_tensor(b_p[li][:npart], mean[:npart], a_p[li][:npart],
                                        ALU.mult)
                nc.vector.tensor_tensor(b_p[li][:npart], bet_s[:npart, li:li + 1],
                                        b_p[li][:npart], ALU.subtract)

            def pack_params(li):
                """replicate a,b [0:64] -> [64:128] for packed 64-ch layers."""
                nc.sync.dma_start(a_p[li][64:128, :], a_p[li][0:64, :])
                nc.sync.dma_start(b_p[li][64:128, :], b_p[li][0:64, :])

            def reduce_pair_and_allreduce(ag, n, idx):
                """packed [128,2] -> fold halves -> AllReduce -> affine."""
                sums = stats_to_sums(ag, n, 128)
                lo = pers.tile([64, 2], f32, name=f"lo{idx}")
                nc.sync.dma_start(lo[:], sums[64:128, :])
                nc.vector.tensor_tensor(sums[0:64, :], sums[0:64, :], lo[:], ALU.add)
                nc.sync.dma_start(pay_i[idx][0:128].rearrange("(p c) -> p c", c=2),
                                  sums[0:64, :])
                do_allreduce(idx)
                back = pers.tile([128, 2], f32, name=f"backp{idx}")
                nc.sync.dma_start(back[0:64, :],
                                  pay_o[idx][0:128].rearrange("(p c) -> p c", c=2))
                affine_from_sums(back, idx, 64, N_GLOBAL)
                pack_params(idx)

            def full_allreduce(ag, n, idx):
                sums = stats_to_sums(ag, n, 128)
                nc.sync.dma_start(pay_i[idx][0:256].rearrange("(p c) -> p c", c=2),
                                  sums[:])
                do_allreduce(idx)
                back = pers.tile([128, 2], f32, name=f"backf{idx}")
                nc.sync.dma_start(back[:],
                                  pay_o[idx][0:256].rearrange("(p c) -> p c", c=2))
                affine_from_sums(back, idx, 128, N_GLOBAL)

            # ============ phase 1: conv1 + x1 stats + xyz prep ===========
            with tc.tile_pool(name="p1", bufs=1) as p1, \
                 tc.tile_pool(name="p1s", bufs=3) as p1s, \
                 tc.tile_pool(name="ps1", bufs=2, space="PSUM") as ps1, \
                 tc.tile_pool(name="ps1t", bufs=2, space="PSUM") as ps1t:

                # --- lc_featT: [G,64] -> [64,G] fp16 via PE transpose ----
                for i in range(G // 128):
                    lf = p1s.tile([128, 64], f32, name="lf")
                    nc.sync.dma_start(lf[:], lc_feat[128 * i:128 * (i + 1), :])
                    lfh = p1s.tile([128, 64], f16, name="lfh")
                    nc.vector.tensor_copy(lfh[:], lf[:])
                    ptr = ps1t.tile([64, 128], f16, name="ptr")
                    nc.tensor.transpose(ptr[:], lfh[:], ident[:])
                    nc.scalar.copy(lcT[:, 128 * i:128 * (i + 1)], ptr[:])

                # --- xyz: rel0, moments, A/Bv/Cg (points-major) ----------
                xyz = p1.tile([128, 1536], f32, name="xyz")
                nc.sync.dma_start(xyz[:], knn_xyz[:])
                lcs = p1.tile([128, 48], f32, name="lcs")
                nc.sync.dma_start(lcs[:], lc_small[:])
                rel0 = p1.tile([128, 1536], f32, name="rel0")
                lc_b = lcs[:].rearrange("p (g c) -> p g c", c=3).unsqueeze(2) \
                    .broadcast_to([128, 16, 32, 3])
                nc.vector.tensor_tensor(
                    rel0[:].rearrange("p (g k c) -> p g k c", k=32, c=3),
                    xyz[:].rearrange("p (g k c) -> p g k c", k=32, c=3),
                    lc_b, ALU.subtract)
                sq = p1.tile([128, 1536], f32, name="sq")
                nc.vector.tensor_tensor(sq[:], rel0[:], rel0[:], ALU.mult)
                A_ = p1.tile([128, 512], f32, name="A_")
                nc.vector.tensor_reduce(
                    A_[:], sq[:].rearrange("p (n c) -> p n c", c=3), AX.X, ALU.add)
                s2part = p1.tile([128, 1], f32, name="s2part")
                nc.vector.tensor_reduce(s2part[:], sq[:], AX.X, ALU.add)
                s1part = p1.tile([128, 1], f32, name="s1part")
                nc.vector.tensor_reduce(s1part[:], rel0[:], AX.X, ALU.add)
                bv_t = p1.tile([128, 1536], f32, name="bv_t", tag="sq")
                nc.vector.tensor_tensor(
                    bv_t[:].rearrange("p (g k c) -> p g k c", k=32, c=3),
                    rel0[:].rearrange("p (g k c) -> p g k c", k=32, c=3),
                    lc_b, ALU.mult)
                Bv = p1.tile([128, 512], f32, name="Bv")
                nc.vector.tensor_reduce(
                    Bv[:], bv_t[:].rearrange("p (n c) -> p n c", c=3), AX.X, ALU.add)
                lsq = p1.tile([128, 48], f32, name="lsq")
                nc.vector.tensor_tensor(lsq[:], lcs[:], lcs[:], ALU.mult)
                Cg = p1.tile([128, 16], f32, name="Cg")
                nc.vector.tensor_reduce(
                    Cg[:], lsq[:].rearrange("p (g c) -> p g c", c=3), AX.X, ALU.add)

                # --- main conv1 loop -------------------------------------
                for i in range(NT):
                    stg = p1s.tile([128, 4 * 67], f16, name="stg")
                    nc.gpsimd.dma_start(
                        stg[:].rearrange("p (s c) -> p s c", c=67),
                        knn_feat[:].rearrange("(s p) c -> p s c", p=128)
                        [:, 4 * i:4 * (i + 1), :])
                    etp = ps1t.tile([67, 512], f16, name="etp")
                    for s in range(4):
                        nc.tensor.transpose(
                            etp[:, 128 * s:128 * (s + 1)],
                            stg[:, 67 * s:67 * (s + 1)], ident[:])
                    e_t = p1s.tile([67, 512], f16, name="e_t")
                    nc.scalar.copy(e_t[:], etp[:])
                    xp = ps1.tile([128, 512], f32, name="xp")
                    nc.tensor.matmul(xp[:], w1a_s[:], e_t[:], start=True, stop=False)
                    nc.tensor.matmul(
                        xp[:], w1b_s[:],
                        lcT[:, 16 * i:16 * (i + 1)].unsqueeze(2)
                        .broadcast_to([64, 16, 32]),
                        start=False, stop=True)
                    nc.vector.bn_stats(st[:, i, :], xp[:])
                    nc.scalar.copy(x_slot[:, NP * i:NP * (i + 1)], xp[:])

                # --- AR1: x1 stats + rel0 moments ------------------------
                ag = p1.tile([128, 2], f32, name="ag")
                nc.vector.bn_aggr(ag[:], st[:])
                sums = stats_to_sums(ag, P, 128)
                nc.sync.dma_start(pay_i[0][0:256].rearrange("(p c) -> p c", c=2), sums[:])
                nc.sync.dma_start(pay_i[0][256:384].rearrange("(p c) -> p c", c=1), s2part[:])
                nc.sync.dma_start(pay_i[0][384:512].rearrange("(p c) -> p c", c=1), s1part[:])
                do_allreduce(0)
                back = p1.tile([128, 2], f32, name="back")
                nc.sync.dma_start(back[:], pay_o[0][0:256].rearrange("(p c) -> p c", c=2))
                affine_from_sums(back, 0, 128, N_GLOBAL)
                s2row = p1.tile([1, 128], f32, name="s2row")
                nc.sync.dma_start(s2row[:], pay_o[0][256:384].rearrange("(c n) -> c n", c=1))
                s1row = p1.tile([1, 128], f32, name="s1row")
                nc.sync.dma_start(s1row[:], pay_o[0][384:512].rearrange("(c n) -> c n", c=1))
                s2 = p1.tile([1, 1], f32, name="s2")
                nc.vector.tensor_reduce(s2[:], s2row[:], AX.X, ALU.add)
                s1 = p1.tile([1, 1], f32, name="s1")
                nc.vector.tensor_reduce(s1[:], s1row[:], AX.X, ALU.add)
                # std = sqrt((S2 - S1^2/N3)/(N3-1)) + 1e-5   (ddof=1)
                mrel = p1.tile([1, 1], f32, name="mrel")
                nc.scalar.mul(mrel[:], s1[:], 1.0 / N3)
                nc.vector.tensor_tensor(mrel[:], mrel[:], s1[:], ALU.mult)
                nc.vector.tensor_tensor(mrel[:], s2[:], mrel[:], ALU.subtract)
                stdv = p1.tile([1, 1], f32, name="stdv")
                nc.scalar.activation(stdv[:], mrel[:], AF.Sqrt, scale=1.0 / (N3 - 1))
                nc.scalar.activation(stdv[:], stdv[:], AF.Identity, bias=c_eps[0:1])
                rstd = p1.tile([1, 1], f32, name="rstd")
                nc.vector.reciprocal(rstd[:], stdv[:])
                rstd_b = p1.tile([128, 1], f32, name="rstd_b")
                nc.gpsimd.partition_broadcast(rstd_b[:], rstd[:])
                rstd2_b = p1.tile([128, 1], f32, name="rstd2_b")
                nc.vector.tensor_tensor(rstd2_b[:], rstd_b[:], rstd_b[:], ALU.mult)
                n2rstd_b = p1.tile([128, 1], f32, name="n2rstd_b")
                nc.scalar.mul(n2rstd_b[:], rstd_b[:], -2.0)

                # d2 = rstd^2*A - 2*rstd*Bv + Cg(bcast); w = exp(-sqrt(d2)/2)
                d2 = p1.tile([128, 512], f32, name="d2", tag="xyz")
                nc.vector.scalar_tensor_tensor(
                    d2[:].rearrange("p (g k) -> p g k", k=32),
                    Bv[:].rearrange("p (g k) -> p g k", k=32), n2rstd_b[:],
                    Cg[:].unsqueeze(2).broadcast_to([128, 16, 32]),
                    ALU.mult, ALU.add)
                nc.vector.scalar_tensor_tensor(
                    d2[:], A_[:], rstd2_b[:], d2[:], ALU.mult, ALU.add)
                distt = p1.tile([128, 512], f32, name="distt", tag="A_")
                nc.scalar.activation(distt[:], d2[:], AF.Sqrt)
                w_pm = p1.tile([128, 512], f16, name="w_pm")
                nc.scalar.activation(w_pm[:], distt[:], AF.Exp, scale=-0.5)
                nc.sync.dma_start(w_row[:].rearrange("(p n) -> p n", n=512), w_pm[:])

            # ============ phase 2: xw + h0 + dn0 stats ===================
            with tc.tile_pool(name="p2w", bufs=4) as p2w, \
                 tc.tile_pool(name="p2s", bufs=3) as p2s, \
                 tc.tile_pool(name="ps2w", bufs=4, space="PSUM") as ps2w, \
                 tc.tile_pool(name="ps2h", bufs=2, space="PSUM") as ps2h:

                def make_xw(t):
                    """x_slot tile t: x1 -> relu(a1*x1+b1)*w (in place)."""
                    cols = slice(NP * t, NP * (t + 1))
                    wt = p2w.tile([1, 512], f16, name="wt")
                    nc.sync.dma_start(
                        wt[:], w_row[NP * t:NP * (t + 1)].rearrange("(c n) -> c n", c=1))
                    wb = ps2w.tile([128, 512], f32, name="wb")
                    nc.tensor.matmul(wb[:], ones1[:], wt[:], start=True, stop=True)
                    xnr = p2s.tile([128, 512], f16, name="xnr")
                    nc.scalar.activation(xnr[:], x_slot[:, cols], AF.Relu,
                                         bias=b_p[0][:], scale=a_p[0][:])
                    nc.vector.tensor_tensor(x_slot[:, cols], xnr[:], wb[:], ALU.mult)

                for j in range(NJ):
                    make_xw(j)
                    make_xw(j + NJ)
                    hp = ps2h.tile([128, 512], f32, name="hp")
                    nc.tensor.matmul(hp[0:64, :], wd_s[0][:],
                                     x_slot[:, NP * j:NP * (j + 1)],
                                     start=True, stop=True, tile_position=(0, 0))
                    nc.tensor.matmul(hp[64:128, :], wd_s[0][:],
                                     x_slot[:, NP * (j + NJ):NP * (j + NJ + 1)],
                                     start=True, stop=True, tile_position=(0, 64))
                    nc.vector.bn_stats(st[:, j, :], hp[:])
                    hst = p2s.tile([128, 512], f16, name="hst")
                    nc.scalar.copy(hst[:], hp[:])
                    nc.sync.dma_start(h_dram[:, NP * j:NP * (j + 1)], hst[:])

                ag2 = p2s.tile([128, 2], f32, name="ag2")
                nc.vector.bn_aggr(ag2[:], st[:, 0:NJ, :])
                reduce_pair_and_allreduce(ag2, HALF, 1)

            # ============ phase 3: t + u0 stats ==========================
            with tc.tile_pool(name="p3s", bufs=3) as p3s, \
                 tc.tile_pool(name="ps3", bufs=2, space="PSUM") as ps3:
                for j in range(NJ):
                    cols = slice(NP * j, NP * (j + 1))
                    hin = p3s.tile([128, 512], f16, name="hin")
                    nc.sync.dma_start(hin[:], h_dram[:, cols])
                    tst = p3s.tile([128, 512], f16, name="tst")
                    nc.scalar.activation(tst[:], hin[:], AF.Relu,
                                         bias=b_p[1][:], scale=a_p[1][:])
                    nc.sync.dma_start(t_dram[:, cols], tst[:])
                    upA = ps3.tile([128, 512], f32, name="upA")
                    nc.tensor.matmul(upA[:], wu_s[0][0:64, :], tst[0:64, :],
                                     start=True, stop=True)
                    upB = ps3.tile([128, 512], f32, name="upB")
                    nc.tensor.matmul(upB[:], wu_s[0][64:128, :], tst[64:128, :],
                                     start=True, stop=True)
                    nc.vector.bn_stats(st[:, 2 * j, :], upA[:])
                    nc.vector.bn_stats(st[:, 2 * j + 1, :], upB[:])

                ag3 = p3s.tile([128, 2], f32, name="ag3")
                nc.vector.bn_aggr(ag3[:], st[:])
                full_allreduce(ag3, P, 2)

            # ============ phase 4: r1 + h1 + dn1 stats ===================
            with tc.tile_pool(name="p4s", bufs=4) as p4s, \
                 tc.tile_pool(name="ps4u", bufs=2, space="PSUM") as ps4u, \
                 tc.tile_pool(name="ps4h", bufs=2, space="PSUM") as ps4h:

                def resid(up, t):
                    """x_slot tile t: xw -> relu((a2*u+b2) + xw) (in place)."""
                    cols = slice(NP * t, NP * (t + 1))
                    bnu = p4s.tile([128, 512], f16, name="bnu")
                    nc.scalar.activation(bnu[:], up[:], AF.Identity,
                                         bias=b_p[2][:], scale=a_p[2][:])
                    nc.vector.tensor_tensor(bnu[:], bnu[:], x_slot[:, cols], ALU.add)
                    nc.vector.tensor_scalar_max(x_slot[:, cols], bnu[:], 0.0)

                for j in range(NJ):
                    cols = slice(NP * j, NP * (j + 1))
                    tin = p4s.tile([128, 512], f16, name="tin")
                    nc.sync.dma_start(tin[:], t_dram[:, cols])
                    upA = ps4u.tile([128, 512], f32, name="upA4")
                    nc.tensor.matmul(upA[:], wu_s[0][0:64, :], tin[0:64, :],
                                     start=True, stop=True)
                    upB = ps4u.tile([128, 512], f32, name="upB4")
                    nc.tensor.matmul(upB[:], wu_s[0][64:128, :], tin[64:128, :],
                                     start=True, stop=True)
                    resid(upA, j)
                    resid(upB, j + NJ)
                    hp = ps4h.tile([128, 512], f32, name="hp4")
                    nc.tensor.matmul(hp[0:64, :], wd_s[1][:],
                                     x_slot[:, NP * j:NP * (j + 1)],
                                     start=True, stop=True, tile_position=(0, 0))
                    nc.tensor.matmul(hp[64:128, :], wd_s[1][:],
                                     x_slot[:, NP * (j + NJ):NP * (j + NJ + 1)],
                                     start=True, stop=True, tile_position=(0, 64))
                    nc.vector.bn_stats(st[:, j, :], hp[:])
                    hst4 = p4s.tile([128, 512], f16, name="hst4")
                    nc.scalar.copy(hst4[:], hp[:])
                    nc.sync.dma_start(h_dram[:, cols], hst4[:])

                ag4 = p4s.tile([128, 2], f32, name="ag4")
                nc.vector.bn_aggr(ag4[:], st[:, 0:NJ, :])
                reduce_pair_and_allreduce(ag4, HALF, 3)

            # ============ phase 5: t1 + u1 stats =========================
            with tc.tile_pool(name="p5s", bufs=3) as p5s, \
                 tc.tile_pool(name="ps5", bufs=2, space="PSUM") as ps5:
                for j in range(NJ):
                    cols = slice(NP * j, NP * (j + 1))
                    hin5 = p5s.tile([128, 512], f16, name="hin5")
                    nc.sync.dma_start(hin5[:], h_dram[:, cols])
                    tst5 = p5s.tile([128, 512], f16, name="tst5")
                    nc.scalar.activation(tst5[:], hin5[:], AF.Relu,
                                         bias=b_p[3][:], scale=a_p[3][:])
                    nc.sync.dma_start(t_dram[:, cols], tst5[:])
                    upA = ps5.tile([128, 512], f32, name="upA5")
                    nc.tensor.matmul(upA[:], wu_s[1][0:64, :], tst5[0:64, :],
                                     start=True, stop=True)
                    upB = ps5.tile([128, 512], f32, name="upB5")
                    nc.tensor.matmul(upB[:], wu_s[1][64:128, :], tst5[64:128, :],
                                     start=True, stop=True)
                    nc.vector.bn_stats(st[:, 2 * j, :], upA[:])
                    nc.vector.bn_stats(st[:, 2 * j + 1, :], upB[:])

                ag5 = p5s.tile([128, 2], f32, name="ag5")
                nc.vector.bn_aggr(ag5[:], st[:])
                full_allreduce(ag5, P, 4)

            # ============ phase 6: final =================================
            with tc.tile_pool(name="p6s", bufs=4) as p6s, \
                 tc.tile_pool(name="ps6", bufs=2, space="PSUM") as ps6:

                def final(up, t):
                    cols = slice(NP * t, NP * (t + 1))
                    bnu = p6s.tile([128, 512], f32, name="bnu6")
                    nc.scalar.activation(bnu[:], up[:], AF.Identity,
                                         bias=b_p[4][:], scale=a_p[4][:])
                    ot = p6s.tile([128, 512], f32, name="ot")
                    nc.vector.tensor_tensor(ot[:], bnu[:], x_slot[:, cols], ALU.add)
                    nc.vector.tensor_scalar_max(ot[:], ot[:], 0.0)
                    nc.sync.dma_start(out[:, cols], ot[:])

                for j in range(NJ):
                    cols = slice(NP * j, NP * (j + 1))
                    tin6 = p6s.tile([128, 512], f16, name="tin6")
                    nc.sync.dma_start(tin6[:], t_dram[:, cols])
                    upA = ps6.tile([128, 512], f32, name="upA6")
                    nc.tensor.matmul(upA[:], wu_s[1][0:64, :], tin6[0:64, :],
                                     start=True, stop=True)
                    upB = ps6.tile([128, 512], f32, name="upB6")
                    nc.tensor.matmul(upB[:], wu_s[1][64:128, :], tin6[64:128, :],
                                     start=True, stop=True)
                    final(upA, j)
                    final(upB, j + NJ)

    nc.compile()
    return nc


def _prep_inputs(lc_xyz, lc_feat, knn_xyz, knn_feat, w1, bn1_g, bn1_b,
                 wd, bd, dn_g, dn_b, wu, bu, up_g, up_b):
    f16 = np.float16
    w1aT = np.ascontiguousarray(w1[:, :67].T).astype(f16)
    w1bT = np.ascontiguousarray(w1[:, 67:].T).astype(f16)
    wdT = np.ascontiguousarray(np.transpose(wd, (0, 2, 1))).astype(f16)  # [2,128,64]
    wuT = np.ascontiguousarray(np.transpose(wu, (0, 2, 1))).astype(f16)  # [2,64,128]
    gam = np.zeros((5, 128), np.float32)
    bet = np.zeros((5, 128), np.float32)
    gam[0], bet[0] = bn1_g, bn1_b
    gam[1, :64], bet[1, :64] = dn_g[0], dn_b[0]
    gam[2], bet[2] = up_g[0], up_b[0]
    gam[3, :64], bet[3, :64] = dn_g[1], dn_b[1]
    gam[4], bet[4] = up_g[1], up_b[1]
    shared = dict(w1aT=w1aT, w1bT=w1bT, wdT=wdT, wuT=wuT, gam=gam, bet=bet)
    in_maps = []
    for b in range(B):
        m = dict(shared)
        m["knn_feat"] = np.ascontiguousarray(knn_feat[b].reshape(P, 67))
        m["knn_xyz"] = np.ascontiguousarray(knn_xyz[b].reshape(128, 1536))
        m["lc_small"] = np.ascontiguousarray(lc_xyz[b].reshape(128, 48))
        m["lc_feat"] = np.ascontiguousarray(lc_feat[b])
        in_maps.append(m)
    return in_maps


def get_nc():
    if "nc" not in _CACHE:
        _CACHE["nc"] = _build(8)
    return _CACHE["nc"]


def make_runner(nc, n_cores=8):
    """Build the shard_map'd executable once; returns (run, in_names).

    Modeled on bass2jax.run_bass_via_pjrt, but caches the jitted callable
    so repeated invocations don't re-trace (needed for timing loops).
    """
    import jax
    from jax.sharding import Mesh, PartitionSpec
    from jax.experimental.shard_map import shard_map
    from concourse import bass2jax
    from concourse import mybir as _mybir

    bass2jax.install_neuronx_cc_hook()
    partition_name = nc.partition_id_tensor.name if nc.partition_id_tensor else None
    in_names, out_names, out_avals, zero_outs = [], [], [], []
    for alloc in nc.m.functions[0].allocations:
        if not isinstance(_mybir.MemoryLocationSet, type) or not isinstance(
                alloc, _mybir.MemoryLocationSet):
            continue
        name = alloc.memorylocations[0].name
        if alloc.kind == "ExternalInput":
            if name != partition_name:
                in_names.append(name)
        elif alloc.kind == "ExternalOutput":
            out_names.append(name)
            shape = tuple(alloc.tensor_shape)
            dtype = _mybir.dt.np(alloc.dtype)
            out_avals.append(jax.core.ShapedArray(shape, dtype))
            zero_outs.append(np.zeros(shape, dtype))
    n_params = len(in_names)
    all_names = in_names + out_names
    if partition_name is not None:
        all_names = all_names + [partition_name]

    def _body(*args):
        operands = list(args)
        if partition_name is not None:
            operands.append(bass2jax.partition_id_tensor())
        outs = bass2jax._bass_exec_p.bind(
            *operands,
            out_avals=tuple(out_avals),
            in_names=tuple(all_names),
            out_names=tuple(out_names),
            lowering_input_output_aliases=(),
            sim_require_finite=True,
            sim_require_nnan=True,
            nc=nc,
        )
        return tuple(outs)

    devices = jax.devices()[:n_cores]
    mesh = Mesh(np.asarray(devices), ("core",))
    n_outs = len(out_names)
    sharded = jax.jit(
        shard_map(_body, mesh=mesh,
                  in_specs=(PartitionSpec("core"),) * (n_params + n_outs),
                  out_specs=(PartitionSpec("core"),) * n_outs,
                  check_rep=False),
        donate_argnums=tuple(range(n_params, n_params + n_outs)),
        keep_unused=True)

    def run(in_maps, timing_reps=0):
        concat_in = [np.concatenate([np.asarray(in_maps[c][k])[None]
                                     for c in range(n_cores)], axis=0)
                     .reshape(n_cores * in_maps[0][k].shape[0],
                              *in_maps[0][k].shape[1:])
                     for k in in_names]
        concat_zeros = [np.zeros((n_cores * z.shape[0], *z.shape[1:]), z.dtype)
                        for z in zero_outs]
        out_arrs = sharded(*concat_in, *concat_zeros)
        jax.block_until_ready(out_arrs)
        times = []
        if timing_reps:
            import time
            ins_dev = jax.device_put(concat_in)
            jax.block_until_ready(ins_dev)
            for _ in range(timing_reps):
                zer_dev = jax.device_put(concat_zeros)
                jax.block_until_ready(zer_dev)
                t0 = time.perf_counter()
                o = sharded(*ins_dev, *zer_dev)
                jax.block_until_ready(o)
                times.append(time.perf_counter() - t0)
        return ({name: np.asarray(out_arrs[i]).reshape(n_cores, *out_avals[i].shape)
                 for i, name in enumerate(out_names)}, times)

    return run


def kernel(**inputs):
    inputs = {k: np.asarray(v) for k, v in inputs.items()}
    nc = get_nc()
    in_maps = _prep_inputs(**inputs)
    res = bass_utils.run_bass_kernel_spmd(nc, in_maps, core_ids=list(range(8)))
    outs = [res.results[c]["out"].reshape(128, G, KNN) for c in range(B)]
    return np.stack(outs, axis=0)


if __name__ == "__main__":
    import reference
    import jax.numpy as jnp
    inp = {k: np.asarray(v) for k, v in reference.setup_inputs().items()}
    got = kernel(**inp)
    exp = np.asarray(reference.reference(**{k: jnp.asarray(v) for k, v in inp.items()}))
    rel = np.linalg.norm(got - exp) / np.linalg.norm(exp)
    print("Relative error:", rel, "absmax:", np.abs(got - exp).max())



# revision 5
# speedup vs baseline: 190.8376x; 190.8376x over previous
"""Trainium2 Bass kernel for nn_LocalGeoAgg (gnn_message_passing).

Strategy: data-parallel over batch B=8 across the 8 NeuronCores (one
sample per core). All convs are 1x1 so everything is per-point except
the training-mode BatchNorm statistics (and the global std of rel0),
which are all-reduced across cores (sync-BN) with 5 small AllReduces.

Layout: channels on partitions, points (G*K = 65536) on the free dim.
The residual stream x lives in SBUF as fp16 for the whole kernel
(128 KiB/partition); nothing else is staged to DRAM — the 64-channel
h = Wd@x intermediates are recomputed from x when needed (tensor
engine has slack), which removes all intermediate HBM traffic.

Host-side prep stages knn_feat channel-major in fp16 ([67, P]) so
conv1 needs no on-device transposes or casting DMAs, and the output
is written fp16 and upcast on the host.

Conv biases bd/bu are dropped: training-mode BN subtracts the batch
mean, which cancels any per-channel additive constant exactly.
"""

import sys

sys.path.insert(0, "/opt/trn_rl_repo")

import contextlib

import numpy as np

from concourse import bacc, bass, mybir, tile
from concourse import bass_utils

dt = mybir.dt
AF = mybir.ActivationFunctionType
ALU = mybir.AluOpType
AX = mybir.AxisListType

B, G, KNN = 8, 2048, 32
P = G * KNN            # 65536 points per core
NP = 512               # points per tile
NT = P // NP           # 128 tiles
NJ = NT // 2           # 64 coupled (A, B) tile pairs
HALF = P // 2          # 32768
CH = 4096              # input/output DMA chunk (points)
NCH = P // CH          # 16 chunks
EPS = 1e-5
N_GLOBAL = B * P       # BN normalization count
N3 = B * P * 3         # rel0 element count (std)

_CACHE = {}


def _build(n_cores=8, use_cc=True):
    nc = bacc.Bacc("TRN2", target_bir_lowering=False, debug=False,
                   num_devices=n_cores)

    f32, f16 = dt.float32, dt.float16

    # ---- per-core external inputs -------------------------------------
    knn_featT = nc.dram_tensor("knn_featT", [67, P], f16, kind="ExternalInput").ap()
    knn_xyz = nc.dram_tensor("knn_xyz", [128, 1536], f32, kind="ExternalInput").ap()
    lc_small = nc.dram_tensor("lc_small", [128, 48], f32, kind="ExternalInput").ap()
    lc_featT = nc.dram_tensor("lc_featT", [64, G], f16, kind="ExternalInput").ap()
    w1aT = nc.dram_tensor("w1aT", [67, 128], f16, kind="ExternalInput").ap()
    w1bT = nc.dram_tensor("w1bT", [64, 128], f16, kind="ExternalInput").ap()
    wdT = nc.dram_tensor("wdT", [2, 128, 64], f16, kind="ExternalInput").ap()
    wuT = nc.dram_tensor("wuT", [2, 64, 128], f16, kind="ExternalInput").ap()
    gam = nc.dram_tensor("gam", [5, 128], f32, kind="ExternalInput").ap()
    bet = nc.dram_tensor("bet", [5, 128], f32, kind="ExternalInput").ap()
    out = nc.dram_tensor("out", [128, P], f16, kind="ExternalOutput").ap()

    rg = [list(range(n_cores))]

    with tile.TileContext(nc) as tc:
        with contextlib.ExitStack() as stack:
            pers = stack.enter_context(tc.tile_pool(name="pers", bufs=1))
            dram = stack.enter_context(tc.tile_pool(name="dram", bufs=1, space="DRAM"))

            # persistent SBUF residents
            x_slot = pers.tile([128, P], f16, name="x_slot")
            lcT = pers.tile([64, G], f16, name="lcT")
            st = pers.tile([128, NT, 6], f32, name="st")

            # small weights / params
            w1a_s = pers.tile([67, 128], f16, name="w1a_s")
            w1b_s = pers.tile([64, 128], f16, name="w1b_s")
            wd_s = [pers.tile([128, 64], f16, name=f"wd_s{i}") for i in range(2)]
            wu_s = [pers.tile([128, 128], f16, name=f"wu_s{i}") for i in range(2)]
            nc.sync.dma_start(w1a_s[:], w1aT[:])
            nc.sync.dma_start(w1b_s[:], w1bT[:])
            nc.sync.dma_start(lcT[:], lc_featT[:])
            for i in range(2):
                nc.sync.dma_start(wd_s[i][:], wdT[i])
                # up weights: rows 0-63 AND rows 64-127 (row tiling pair)
                nc.sync.dma_start(wu_s[i][0:64, :], wuT[i])
                nc.sync.dma_start(wu_s[i][64:128, :], wuT[i])

            a_p = [pers.tile([128, 1], f32, name=f"a_p{i}") for i in range(5)]
            b_p = [pers.tile([128, 1], f32, name=f"b_p{i}") for i in range(5)]
            c_eps = pers.tile([128, 1], f32, name="c_eps")
            nc.vector.memset(c_eps[:], EPS)
            gam_s = pers.tile([128, 5], f32, name="gam_s")
            bet_s = pers.tile([128, 5], f32, name="bet_s")
            nc.sync.dma_start(gam_s[:], gam[:].rearrange("l c -> c l"))
            nc.sync.dma_start(bet_s[:], bet[:].rearrange("l c -> c l"))

            w_row = dram.tile([P], f16, name="w_row")

            def do_allreduce(idx):
                if use_cc:
                    nc.gpsimd.collective_compute(
                        "AllReduce", ALU.add, ins=[pay_i[idx].opt()],
                        outs=[pay_o[idx].opt()], replica_groups=rg)
                else:
                    nc.sync.dma_start(pay_o[idx][:], pay_i[idx][:])
            pay_sz = [512, 128, 256, 128, 256]
            pay_i = [dram.tile([pay_sz[i]], f32, name=f"pay_i{i}") for i in range(5)]
            pay_o = [dram.tile([pay_sz[i]], f32, name=f"pay_o{i}") for i in range(5)]

            # ---------- helpers ------------------------------------------
            def stats_to_sums(ag, n, npart):
                """[npart,2] (mean,var) -> (sum, sumsq)."""
                i = stats_to_sums.i = stats_to_sums.i + 1
                sums = pers.tile([128, 2], f32, name=f"sums{i}")
                m2 = pers.tile([128, 1], f32, name=f"m2_{i}")
                nc.vector.tensor_tensor(m2[:npart], ag[:npart, 0:1], ag[:npart, 0:1], ALU.mult)
                nc.scalar.mul(sums[:npart, 0:1], ag[:npart, 0:1], float(n))
                nc.vector.tensor_tensor(sums[:npart, 1:2], ag[:npart, 1:2], m2[:npart], ALU.add)
                nc.scalar.mul(sums[:npart, 1:2], sums[:npart, 1:2], float(n))
                return sums

            stats_to_sums.i = 0

            def affine_from_sums(back, li, npart, n_total):
                """back [npart,2] global (sum,sumsq) -> a_p[li], b_p[li]."""
                mean = pers.tile([128, 1], f32, name=f"mean{li}")
                var = pers.tile([128, 1], f32, name=f"var{li}")
                m2 = pers.tile([128, 1], f32, name=f"m2g{li}")
                sig = pers.tile([128, 1], f32, name=f"sig{li}")
                nc.scalar.mul(mean[:npart], back[:npart, 0:1], 1.0 / n_total)
                nc.vector.tensor_tensor(m2[:npart], mean[:npart], mean[:npart], ALU.mult)
                nc.vector.scalar_tensor_tensor(
                    var[:npart], back[:npart, 1:2], 1.0 / n_total, m2[:npart],
                    ALU.mult, ALU.subtract)
                nc.scalar.activation(sig[:npart], var[:npart], AF.Sqrt, bias=c_eps[:npart])
                nc.vector.reciprocal(sig[:npart], sig[:npart])
                nc.vector.tensor_tensor(a_p[li][:npart], gam_s[:npart, li:li + 1],
                                        sig[:npart], ALU.mult)
                nc.vector.tensor_tensor(b_p[li][:npart], mean[:npart], a_p[li][:npart],
                                        ALU.mult)
                nc.vector.tensor_tensor(b_p[li][:npart], bet_s[:npart, li:li + 1],
                                        b_p[li][:npart], ALU.subtract)

            def pack_params(li):
                """replicate a,b [0:64] -> [64:128] for packed 64-ch layers."""
                nc.sync.dma_start(a_p[li][64:128, :], a_p[li][0:64, :])
                nc.sync.dma_start(b_p[li][64:128, :], b_p[li][0:64, :])

            def reduce_pair_and_allreduce(ag, n, idx):
                """packed [128,2] -> fold halves -> AllReduce -> affine."""
                sums = stats_to_sums(ag, n, 128)
                lo = pers.tile([64, 2], f32, name=f"lo{idx}")
                nc.sync.dma_start(lo[:], sums[64:128, :])
                nc.vector.tensor_tensor(sums[0:64, :], sums[0:64, :], lo[:], ALU.add)
                nc.sync.dma_start(pay_i[idx][0:128].rearrange("(p c) -> p c", c=2),
                                  sums[0:64, :])
                do_allreduce(idx)
                back = pers.tile([128, 2], f32, name=f"backp{idx}")
                nc.sync.dma_start(back[0:64, :],
                                  pay_o[idx][0:128].rearrange("(p c) -> p c", c=2))
                affine_from_sums(back, idx, 64, N_GLOBAL)
                pack_params(idx)

            def full_allreduce(ag, n, idx):
                sums = stats_to_sums(ag, n, 128)
                nc.sync.dma_start(pay_i[idx][0:256].rearrange("(p c) -> p c", c=2),
                                  sums[:])
                do_allreduce(idx)
                back = pers.tile([128, 2], f32, name=f"backf{idx}")
                nc.sync.dma_start(back[:],
                                  pay_o[idx][0:256].rearrange("(p c) -> p c", c=2))
                affine_from_sums(back, idx, 128, N_GLOBAL)

            # ============ phase 1: conv1 + x1 stats + xyz prep ===========
            with tc.tile_pool(name="p1", bufs=1) as p1, \
                 tc.tile_pool(name="p1e", bufs=2) as p1e, \
                 tc.tile_pool(name="ps1", bufs=4, space="PSUM") as ps1:

                # --- xyz: rel0, moments, A/Bv/Cg (points-major) ----------
                xyz = p1.tile([128, 1536], f32, name="xyz")
                nc.sync.dma_start(xyz[:], knn_xyz[:])
                lcs = p1.tile([128, 48], f32, name="lcs")
                nc.sync.dma_start(lcs[:], lc_small[:])
                rel0 = p1.tile([128, 1536], f32, name="rel0")
                lc_b = lcs[:].rearrange("p (g c) -> p g c", c=3).unsqueeze(2) \
                    .broadcast_to([128, 16, 32, 3])
                nc.vector.tensor_tensor(
                    rel0[:].rearrange("p (g k c) -> p g k c", k=32, c=3),
                    xyz[:].rearrange("p (g k c) -> p g k c", k=32, c=3),
                    lc_b, ALU.subtract)
                sq = p1.tile([128, 1536], f32, name="sq")
                nc.vector.tensor_tensor(sq[:], rel0[:], rel0[:], ALU.mult)
                A_ = p1.tile([128, 512], f32, name="A_")
                nc.vector.tensor_reduce(
                    A_[:], sq[:].rearrange("p (n c) -> p n c", c=3), AX.X, ALU.add)
                s2part = p1.tile([128, 1], f32, name="s2part")
                nc.vector.tensor_reduce(s2part[:], sq[:], AX.X, ALU.add)
                s1part = p1.tile([128, 1], f32, name="s1part")
                nc.vector.tensor_reduce(s1part[:], rel0[:], AX.X, ALU.add)
                bv_t = p1.tile([128, 1536], f32, name="bv_t", tag="sq")
                nc.vector.tensor_tensor(
                    bv_t[:].rearrange("p (g k c) -> p g k c", k=32, c=3),
                    rel0[:].rearrange("p (g k c) -> p g k c", k=32, c=3),
                    lc_b, ALU.mult)
                Bv = p1.tile([128, 512], f32, name="Bv")
                nc.vector.tensor_reduce(
                    Bv[:], bv_t[:].rearrange("p (n c) -> p n c", c=3), AX.X, ALU.add)
                lsq = p1.tile([128, 48], f32, name="lsq")
                nc.vector.tensor_tensor(lsq[:], lcs[:], lcs[:], ALU.mult)
                Cg = p1.tile([128, 16], f32, name="Cg")
                nc.vector.tensor_reduce(
                    Cg[:], lsq[:].rearrange("p (g c) -> p g c", c=3), AX.X, ALU.add)

                # --- main conv1 loop (chunked direct loads) --------------
                for ch in range(NCH):
                    est = p1e.tile([67, CH], f16, name="est")
                    nc.sync.dma_start(est[:], knn_featT[:, CH * ch:CH * (ch + 1)])
                    for s in range(CH // NP):
                        i = (CH // NP) * ch + s
                        xp = ps1.tile([128, 512], f32, name="xp")
                        nc.tensor.matmul(xp[:], w1a_s[:],
                                         est[:, NP * s:NP * (s + 1)],
                                         start=True, stop=False)
                        nc.tensor.matmul(
                            xp[:], w1b_s[:],
                            lcT[:, 16 * i:16 * (i + 1)].unsqueeze(2)
                            .broadcast_to([64, 16, 32]),
                            start=False, stop=True)
                        nc.scalar.copy(x_slot[:, NP * i:NP * (i + 1)], xp[:])
                        nc.vector.bn_stats(st[:, i, :],
                                           x_slot[:, NP * i:NP * (i + 1)])

                # --- AR1: x1 stats + rel0 moments ------------------------
                ag = p1.tile([128, 2], f32, name="ag")
                nc.vector.bn_aggr(ag[:], st[:])
                sums = stats_to_sums(ag, P, 128)
                nc.sync.dma_start(pay_i[0][0:256].rearrange("(p c) -> p c", c=2), sums[:])
                nc.sync.dma_start(pay_i[0][256:384].rearrange("(p c) -> p c", c=1), s2part[:])
                nc.sync.dma_start(pay_i[0][384:512].rearrange("(p c) -> p c", c=1), s1part[:])
                do_allreduce(0)
                back = p1.tile([128, 2], f32, name="back")
                nc.sync.dma_start(back[:], pay_o[0][0:256].rearrange("(p c) -> p c", c=2))
                affine_from_sums(back, 0, 128, N_GLOBAL)
                s2row = p1.tile([1, 128], f32, name="s2row")
                nc.sync.dma_start(s2row[:], pay_o[0][256:384].rearrange("(c n) -> c n", c=1))
                s1row = p1.tile([1, 128], f32, name="s1row")
                nc.sync.dma_start(s1row[:], pay_o[0][384:512].rearrange("(c n) -> c n", c=1))
                s2 = p1.tile([1, 1], f32, name="s2")
                nc.vector.tensor_reduce(s2[:], s2row[:], AX.X, ALU.add)
                s1 = p1.tile([1, 1], f32, name="s1")
                nc.vector.tensor_reduce(s1[:], s1row[:], AX.X, ALU.add)
                # std = sqrt((S2 - S1^2/N3)/(N3-1)) + 1e-5   (ddof=1)
                mrel = p1.tile([1, 1], f32, name="mrel")
                nc.scalar.mul(mrel[:], s1[:], 1.0 / N3)
                nc.vector.tensor_tensor(mrel[:], mrel[:], s1[:], ALU.mult)
                nc.vector.tensor_tensor(mrel[:], s2[:], mrel[:], ALU.subtract)
                stdv = p1.tile([1, 1], f32, name="stdv")
                nc.scalar.activation(stdv[:], mrel[:], AF.Sqrt, scale=1.0 / (N3 - 1))
                nc.scalar.activation(stdv[:], stdv[:], AF.Identity, bias=c_eps[0:1])
                rstd = p1.tile([1, 1], f32, name="rstd")
                nc.vector.reciprocal(rstd[:], stdv[:])
                rstd_b = p1.tile([128, 1], f32, name="rstd_b")
                nc.gpsimd.partition_broadcast(rstd_b[:], rstd[:])
                rstd2_b = p1.tile([128, 1], f32, name="rstd2_b")
                nc.vector.tensor_tensor(rstd2_b[:], rstd_b[:], rstd_b[:], ALU.mult)
                n2rstd_b = p1.tile([128, 1], f32, name="n2rstd_b")
                nc.scalar.mul(n2rstd_b[:], rstd_b[:], -2.0)

                # d2 = rstd^2*A - 2*rstd*Bv + Cg(bcast); w = exp(-sqrt(d2)/2)
                d2 = p1.tile([128, 512], f32, name="d2", tag="xyz")
                nc.vector.scalar_tensor_tensor(
                    d2[:].rearrange("p (g k) -> p g k", k=32),
                    Bv[:].rearrange("p (g k) -> p g k", k=32), n2rstd_b[:],
                    Cg[:].unsqueeze(2).broadcast_to([128, 16, 32]),
                    ALU.mult, ALU.add)
                nc.vector.scalar_tensor_tensor(
                    d2[:], A_[:], rstd2_b[:], d2[:], ALU.mult, ALU.add)
                distt = p1.tile([128, 512], f32, name="distt", tag="A_")
                nc.scalar.activation(distt[:], d2[:], AF.Sqrt)
                w_pm = p1.tile([128, 512], f16, name="w_pm")
                nc.scalar.activation(w_pm[:], distt[:], AF.Exp, scale=-0.5)
                nc.sync.dma_start(w_row[:].rearrange("(p n) -> p n", n=512), w_pm[:])

            # ============ phase 2: xw + h0 stats =========================
            with tc.tile_pool(name="p2s", bufs=4) as p2s, \
                 tc.tile_pool(name="ps2h", bufs=4, space="PSUM") as ps2h:

                def make_xw(t):
                    """x_slot tile t: x1 -> relu(a1*x1+b1)*w (in place)."""
                    cols = slice(NP * t, NP * (t + 1))
                    wt = p2s.tile([1, 512], f16, name="wt")
                    nc.sync.dma_start(
                        wt[:], w_row[NP * t:NP * (t + 1)].rearrange("(c n) -> c n", c=1))
                    wb = p2s.tile([128, 512], f16, name="wb")
                    nc.gpsimd.partition_broadcast(wb[:], wt[:])
                    xnr = p2s.tile([128, 512], f16, name="xnr")
                    nc.scalar.activation(xnr[:], x_slot[:, cols], AF.Relu,
                                         bias=b_p[0][:], scale=a_p[0][:])
                    nc.vector.tensor_tensor(x_slot[:, cols], xnr[:], wb[:], ALU.mult)

                for j in range(NJ):
                    make_xw(j)
                    make_xw(j + NJ)
                    hp = ps2h.tile([128, 512], f32, name="hp")
                    nc.tensor.matmul(hp[0:64, :], wd_s[0][:],
                                     x_slot[:, NP * j:NP * (j + 1)],
                                     start=True, stop=True, tile_position=(0, 0))
                    nc.tensor.matmul(hp[64:128, :], wd_s[0][:],
                                     x_slot[:, NP * (j + NJ):NP * (j + NJ + 1)],
                                     start=True, stop=True, tile_position=(0, 64))
                    nc.vector.bn_stats(st[:, j, :], hp[:])

                ag2 = p2s.tile([128, 2], f32, name="ag2")
                nc.vector.bn_aggr(ag2[:], st[:, 0:NJ, :])
                reduce_pair_and_allreduce(ag2, HALF, 1)

            # ============ phase 3: h0 recompute + t + u0 stats ===========
            with tc.tile_pool(name="p3s", bufs=3) as p3s, \
                 tc.tile_pool(name="ps3h", bufs=2, space="PSUM") as ps3h, \
                 tc.tile_pool(name="ps3u", bufs=2, space="PSUM") as ps3u:
                for j in range(NJ):
                    hp = ps3h.tile([128, 512], f32, name="hp3")
                    nc.tensor.matmul(hp[0:64, :], wd_s[0][:],
                                     x_slot[:, NP * j:NP * (j + 1)],
                                     start=True, stop=True, tile_position=(0, 0))
                    nc.tensor.matmul(hp[64:128, :], wd_s[0][:],
                                     x_slot[:, NP * (j + NJ):NP * (j + NJ + 1)],
                                     start=True, stop=True, tile_position=(0, 64))
                    tst = p3s.tile([128, 512], f16, name="tst")
                    nc.scalar.activation(tst[:], hp[:], AF.Relu,
                                         bias=b_p[1][:], scale=a_p[1][:])
                    upA = ps3u.tile([128, 512], f32, name="upA")
                    nc.tensor.matmul(upA[:], wu_s[0][0:64, :], tst[0:64, :],
                                     start=True, stop=True)
                    upB = ps3u.tile([128, 512], f32, name="upB")
                    nc.tensor.matmul(upB[:], wu_s[0][64:128, :], tst[64:128, :],
                                     start=True, stop=True)
                    nc.vector.bn_stats(st[:, 2 * j, :], upA[:])
                    nc.vector.bn_stats(st[:, 2 * j + 1, :], upB[:])

                ag3 = p3s.tile([128, 2], f32, name="ag3")
                nc.vector.bn_aggr(ag3[:], st[:])
                full_allreduce(ag3, P, 2)

            # ============ phase 4: r1 + h1 stats =========================
            with tc.tile_pool(name="p4s", bufs=4) as p4s, \
                 tc.tile_pool(name="ps4h", bufs=2, space="PSUM") as ps4h, \
                 tc.tile_pool(name="ps4u", bufs=2, space="PSUM") as ps4u, \
                 tc.tile_pool(name="ps4g", bufs=2, space="PSUM") as ps4g:

                def resid(up, t):
                    """x_slot tile t: xw -> relu((a2*u+b2) + xw) (in place)."""
                    cols = slice(NP * t, NP * (t + 1))
                    tmp = p4s.tile([128, 512], f32, name="tmp4")
                    nc.vector.scalar_tensor_tensor(
                        tmp[:], up[:], a_p[2][:], x_slot[:, cols],
                        ALU.mult, ALU.add)
                    nc.scalar.activation(x_slot[:, cols], tmp[:], AF.Relu,
                                         bias=b_p[2][:])

                for j in range(NJ):
                    hp = ps4h.tile([128, 512], f32, name="hp4")
                    nc.tensor.matmul(hp[0:64, :], wd_s[0][:],
                                     x_slot[:, NP * j:NP * (j + 1)],
                                     start=True, stop=True, tile_position=(0, 0))
                    nc.tensor.matmul(hp[64:128, :], wd_s[0][:],
                                     x_slot[:, NP * (j + NJ):NP * (j + NJ + 1)],
                                     start=True, stop=True, tile_position=(0, 64))
                    tst = p4s.tile([128, 512], f16, name="tst4")
                    nc.scalar.activation(tst[:], hp[:], AF.Relu,
                                         bias=b_p[1][:], scale=a_p[1][:])
                    upA = ps4u.tile([128, 512], f32, name="upA4")
                    nc.tensor.matmul(upA[:], wu_s[0][0:64, :], tst[0:64, :],
                                     start=True, stop=True)
                    upB = ps4u.tile([128, 512], f32, name="upB4")
                    nc.tensor.matmul(upB[:], wu_s[0][64:128, :], tst[64:128, :],
                                     start=True, stop=True)
                    resid(upA, j)
                    resid(upB, j + NJ)
                    hg = ps4g.tile([128, 512], f32, name="hg4")
                    nc.tensor.matmul(hg[0:64, :], wd_s[1][:],
                                     x_slot[:, NP * j:NP * (j + 1)],
                                     start=True, stop=True, tile_position=(0, 0))
                    nc.tensor.matmul(hg[64:128, :], wd_s[1][:],
                                     x_slot[:, NP * (j + NJ):NP * (j + NJ + 1)],
                                     start=True, stop=True, tile_position=(0, 64))
                    nc.vector.bn_stats(st[:, j, :], hg[:])

                ag4 = p4s.tile([128, 2], f32, name="ag4")
                nc.vector.bn_aggr(ag4[:], st[:, 0:NJ, :])
                reduce_pair_and_allreduce(ag4, HALF, 3)

            # ============ phase 5: h1 recompute + t1 + u1 stats ==========
            with tc.tile_pool(name="p5s", bufs=3) as p5s, \
                 tc.tile_pool(name="ps5h", bufs=2, space="PSUM") as ps5h, \
                 tc.tile_pool(name="ps5u", bufs=2, space="PSUM") as ps5u:
                for j in range(NJ):
                    hp = ps5h.tile([128, 512], f32, name="hp5")
                    nc.tensor.matmul(hp[0:64, :], wd_s[1][:],
                                     x_slot[:, NP * j:NP * (j + 1)],
                                     start=True, stop=True, tile_position=(0, 0))
                    nc.tensor.matmul(hp[64:128, :], wd_s[1][:],
                                     x_slot[:, NP * (j + NJ):NP * (j + NJ + 1)],
                                     start=True, stop=True, tile_position=(0, 64))
                    tst = p5s.tile([128, 512], f16, name="tst5")
                    nc.scalar.activation(tst[:], hp[:], AF.Relu,
                                         bias=b_p[3][:], scale=a_p[3][:])
                    upA = ps5u.tile([128, 512], f32, name="upA5")
                    nc.tensor.matmul(upA[:], wu_s[1][0:64, :], tst[0:64, :],
                                     start=True, stop=True)
                    upB = ps5u.tile([128, 512], f32, name="upB5")
                    nc.tensor.matmul(upB[:], wu_s[1][64:128, :], tst[64:128, :],
                                     start=True, stop=True)
                    nc.vector.bn_stats(st[:, 2 * j, :], upA[:])
                    nc.vector.bn_stats(st[:, 2 * j + 1, :], upB[:])

                ag5 = p5s.tile([128, 2], f32, name="ag5")
                nc.vector.bn_aggr(ag5[:], st[:])
                full_allreduce(ag5, P, 4)

            # ============ phase 6: final =================================
            # Output is written per contiguous 4096-point chunk; pair j
            # produces tile j (first half) and tile j+NJ (second half),
            # so chunk c of each half fills from pairs 8c..8c+7.
            with tc.tile_pool(name="p6s", bufs=4) as p6s, \
                 tc.tile_pool(name="p6o", bufs=2) as p6o, \
                 tc.tile_pool(name="ps6h", bufs=2, space="PSUM") as ps6h, \
                 tc.tile_pool(name="ps6u", bufs=2, space="PSUM") as ps6u:

                def final(up, t, ost, s):
                    cols = slice(NP * t, NP * (t + 1))
                    tmp = p6s.tile([128, 512], f32, name="tmp6")
                    nc.vector.scalar_tensor_tensor(
                        tmp[:], up[:], a_p[4][:], x_slot[:, cols],
                        ALU.mult, ALU.add)
                    nc.scalar.activation(ost[:, NP * s:NP * (s + 1)], tmp[:],
                                         AF.Relu, bias=b_p[4][:])

                for c in range(NCH // 2):
                    ostA = p6o.tile([128, CH], f16, name="ostA")
                    ostB = p6o.tile([128, CH], f16, name="ostB")
                    for s in range(CH // NP):
                        j = (CH // NP) * c + s
                        hp = ps6h.tile([128, 512], f32, name="hp6")
                        nc.tensor.matmul(hp[0:64, :], wd_s[1][:],
                                         x_slot[:, NP * j:NP * (j + 1)],
                                         start=True, stop=True, tile_position=(0, 0))
                        nc.tensor.matmul(hp[64:128, :], wd_s[1][:],
                                         x_slot[:, NP * (j + NJ):NP * (j + NJ + 1)],
                                         start=True, stop=True, tile_position=(0, 64))
                        tst = p6s.tile([128, 512], f16, name="tst6")
                        nc.scalar.activation(tst[:], hp[:], AF.Relu,
                                             bias=b_p[3][:], scale=a_p[3][:])
                        upA = ps6u.tile([128, 512], f32, name="upA6")
                        nc.tensor.matmul(upA[:], wu_s[1][0:64, :], tst[0:64, :],
                                         start=True, stop=True)
                        upB = ps6u.tile([128, 512], f32, name="upB6")
                        nc.tensor.matmul(upB[:], wu_s[1][64:128, :], tst[64:128, :],
                                         start=True, stop=True)
                        final(upA, j, ostA, s)
                        final(upB, j + NJ, ostB, s)
                    nc.sync.dma_start(out[:, CH * c:CH * (c + 1)], ostA[:])
                    nc.sync.dma_start(out[:, HALF + CH * c:HALF + CH * (c + 1)],
                                      ostB[:])

    nc.compile()
    return nc


def _prep_inputs(lc_xyz, lc_feat, knn_xyz, knn_feat, w1, bn1_g, bn1_b,
                 wd, bd, dn_g, dn_b, wu, bu, up_g, up_b):
    f16 = np.float16
    w1aT = np.ascontiguousarray(w1[:, :67].T).astype(f16)
    w1bT = np.ascontiguousarray(w1[:, 67:].T).astype(f16)
    wdT = np.ascontiguousarray(np.transpose(wd, (0, 2, 1))).astype(f16)  # [2,128,64]
    wuT = np.ascontiguousarray(np.transpose(wu, (0, 2, 1))).astype(f16)  # [2,64,128]
    gam = np.zeros((5, 128), np.float32)
    bet = np.zeros((5, 128), np.float32)
    gam[0], bet[0] = bn1_g, bn1_b
    gam[1, :64], bet[1, :64] = dn_g[0], dn_b[0]
    gam[2], bet[2] = up_g[0], up_b[0]
    gam[3, :64], bet[3, :64] = dn_g[1], dn_b[1]
    gam[4], bet[4] = up_g[1], up_b[1]
    shared = dict(w1aT=w1aT, w1bT=w1bT, wdT=wdT, wuT=wuT, gam=gam, bet=bet)
    in_maps = []
    for b in range(B):
        m = dict(shared)
        m["knn_featT"] = np.ascontiguousarray(
            knn_feat[b].reshape(P, 67).T.astype(f16))
        m["knn_xyz"] = np.ascontiguousarray(knn_xyz[b].reshape(128, 1536))
        m["lc_small"] = np.ascontiguousarray(lc_xyz[b].reshape(128, 48))
        m["lc_featT"] = np.ascontiguousarray(lc_feat[b].T.astype(f16))
        in_maps.append(m)
    return in_maps


def get_nc():
    if "nc" not in _CACHE:
        _CACHE["nc"] = _build(8)
    return _CACHE["nc"]


def make_runner(nc, n_cores=8):
    """Build the shard_map'd executable once; returns a run callable.

    Modeled on bass2jax.run_bass_via_pjrt, but caches the jitted callable
    so repeated invocations don't re-trace (needed for timing loops).
    """
    import jax
    from jax.sharding import Mesh, PartitionSpec
    from jax.experimental.shard_map import shard_map
    from concourse import bass2jax
    from concourse import mybir as _mybir

    bass2jax.install_neuronx_cc_hook()
    partition_name = nc.partition_id_tensor.name if nc.partition_id_tensor else None
    in_names, out_names, out_avals, zero_outs = [], [], [], []
    for alloc in nc.m.functions[0].allocations:
        if not isinstance(_mybir.MemoryLocationSet, type) or not isinstance(
                alloc, _mybir.MemoryLocationSet):
            continue
        name = alloc.memorylocations[0].name
        if alloc.kind == "ExternalInput":
            if name != partition_name:
                in_names.append(name)
        elif alloc.kind == "ExternalOutput":
            out_names.append(name)
            shape = tuple(alloc.tensor_shape)
            dtype = _mybir.dt.np(alloc.dtype)
            out_avals.append(jax.core.ShapedArray(shape, dtype))
            zero_outs.append(np.zeros(shape, dtype))
    n_params = len(in_names)
    all_names = in_names + out_names
    if partition_name is not None:
        all_names = all_names + [partition_name]

    def _body(*args):
        operands = list(args)
        if partition_name is not None:
            operands.append(bass2jax.partition_id_tensor())
        outs = bass2jax._bass_exec_p.bind(
            *operands,
            out_avals=tuple(out_avals),
            in_names=tuple(all_names),
            out_names=tuple(out_names),
            lowering_input_output_aliases=(),
            sim_require_finite=True,
            sim_require_nnan=True,
            nc=nc,
        )
        return tuple(outs)

    devices = jax.devices()[:n_cores]
    mesh = Mesh(np.asarray(devices), ("core",))
    n_outs = len(out_names)
    sharded = jax.jit(
        shard_map(_body, mesh=mesh,
                  in_specs=(PartitionSpec("core"),) * (n_params + n_outs),
                  out_specs=(PartitionSpec("core"),) * n_outs,
                  check_rep=False),
        donate_argnums=tuple(range(n_params, n_params + n_outs)),
        keep_unused=True)

    def run(in_maps, timing_reps=0):
        concat_in = [np.concatenate([np.asarray(in_maps[c][k])[None]
                                     for c in range(n_cores)], axis=0)
                     .reshape(n_cores * in_maps[0][k].shape[0],
                              *in_maps[0][k].shape[1:])
                     for k in in_names]
        concat_zeros = [np.zeros((n_cores * z.shape[0], *z.shape[1:]), z.dtype)
                        for z in zero_outs]
        out_arrs = sharded(*concat_in, *concat_zeros)
        jax.block_until_ready(out_arrs)
        times = []
        if timing_reps:
            import time
            ins_dev = jax.device_put(concat_in)
            jax.block_until_ready(ins_dev)
            for _ in range(timing_reps):
                zer_dev = jax.device_put(concat_zeros)
                jax.block_until_ready(zer_dev)
                t0 = time.perf_counter()
                o = sharded(*ins_dev, *zer_dev)
                jax.block_until_ready(o)
                times.append(time.perf_counter() - t0)
        return ({name: np.asarray(out_arrs[i]).reshape(n_cores, *out_avals[i].shape)
                 for i, name in enumerate(out_names)}, times)

    return run


def kernel(**inputs):
    inputs = {k: np.asarray(v) for k, v in inputs.items()}
    nc = get_nc()
    in_maps = _prep_inputs(**inputs)
    res = bass_utils.run_bass_kernel_spmd(nc, in_maps, core_ids=list(range(8)))
    outs = [res.results[c]["out"].astype(np.float32).reshape(128, G, KNN)
            for c in range(B)]
    return np.stack(outs, axis=0)


if __name__ == "__main__":
    import reference
    import jax.numpy as jnp
    inp = {k: np.asarray(v) for k, v in reference.setup_inputs().items()}
    got = kernel(**inp)
    exp = np.asarray(reference.reference(**{k: jnp.asarray(v) for k, v in inp.items()}))
    rel = np.linalg.norm(got - exp) / np.linalg.norm(exp)
    print("Relative error:", rel, "absmax:", np.abs(got - exp).max())


# revision 12
# speedup vs baseline: 1248.9141x; 6.5444x over previous
"""Trainium2 Bass kernel for nn_LocalGeoAgg (gnn_message_passing).

Strategy: data-parallel over batch B=8 across the 8 NeuronCores (one
sample per core). All convs are 1x1 so everything is per-point except
the training-mode BatchNorm statistics (and the global std of rel0),
which are all-reduced across cores (sync-BN) with 5 small AllReduces.

Layout: channels on partitions, points (G*K = 65536) on the free dim.
The residual stream x lives in SBUF as fp16 for the whole kernel
(128 KiB/partition); nothing else is staged to DRAM — the 64-channel
h = Wd@x intermediates are recomputed from x when needed (tensor
engine has slack), which removes all intermediate HBM traffic.

Host-side prep stages knn_feat channel-major in fp16 ([67, P]) so
conv1 needs no on-device transposes or casting DMAs, and the output
is written fp16 and upcast on the host.

Conv biases bd/bu are dropped: training-mode BN subtracts the batch
mean, which cancels any per-channel additive constant exactly.
"""

import sys

sys.path.insert(0, "/opt/trn_rl_repo")

import contextlib

import numpy as np

from concourse import bacc, bass, mybir, tile
from concourse import bass_utils

dt = mybir.dt
AF = mybir.ActivationFunctionType
ALU = mybir.AluOpType
AX = mybir.AxisListType

B, G, KNN = 8, 2048, 32
P = G * KNN            # 65536 points per core
NP = 512               # points per tile
NT = P // NP           # 128 tiles
NJ = NT // 2           # 64 coupled (A, B) tile pairs
HALF = P // 2          # 32768
CH = 4096              # input/output DMA chunk (points)
NCH = P // CH          # 16 chunks
EPS = 1e-5
N_GLOBAL = B * P       # BN normalization count
N3 = B * P * 3         # rel0 element count (std)

_CACHE = {}


def _build(n_cores=8, use_cc=True):
    nc = bacc.Bacc("TRN2", target_bir_lowering=False, debug=False,
                   num_devices=n_cores)

    f32, f16 = dt.float32, dt.float16

    # ---- per-core external inputs -------------------------------------
    knn_featT = nc.dram_tensor("knn_featT", [67, P], f16, kind="ExternalInput").ap()
    knn_xyz = nc.dram_tensor("knn_xyz", [128, 1536], f32, kind="ExternalInput").ap()
    lc_small = nc.dram_tensor("lc_small", [128, 48], f32, kind="ExternalInput").ap()
    lc_featT = nc.dram_tensor("lc_featT", [64, G], f16, kind="ExternalInput").ap()
    w1aT = nc.dram_tensor("w1aT", [67, 128], f16, kind="ExternalInput").ap()
    w1bT = nc.dram_tensor("w1bT", [64, 128], f16, kind="ExternalInput").ap()
    wdT = nc.dram_tensor("wdT", [2, 128, 64], f16, kind="ExternalInput").ap()
    wuT = nc.dram_tensor("wuT", [2, 64, 128], f16, kind="ExternalInput").ap()
    gam = nc.dram_tensor("gam", [5, 128], f32, kind="ExternalInput").ap()
    bet = nc.dram_tensor("bet", [5, 128], f32, kind="ExternalInput").ap()
    out = nc.dram_tensor("out", [128, P], f16, kind="ExternalOutput").ap()

    rg = [list(range(n_cores))]

    with tile.TileContext(nc) as tc:
        with contextlib.ExitStack() as stack:
            pers = stack.enter_context(tc.tile_pool(name="pers", bufs=1))
            dram = stack.enter_context(tc.tile_pool(name="dram", bufs=1, space="DRAM"))

            # persistent SBUF residents
            x_slot = pers.tile([128, P], f16, name="x_slot")
            lcT = pers.tile([64, G], f16, name="lcT")
            st = pers.tile([128, NT, 6], f32, name="st")

            # small weights / params
            w1a_s = pers.tile([67, 128], f16, name="w1a_s")
            w1b_s = pers.tile([64, 128], f16, name="w1b_s")
            wd_s = [pers.tile([128, 64], f16, name=f"wd_s{i}") for i in range(2)]
            wu_s = [pers.tile([128, 128], f16, name=f"wu_s{i}") for i in range(2)]
            nc.sync.dma_start(w1a_s[:], w1aT[:])
            nc.sync.dma_start(w1b_s[:], w1bT[:])
            nc.sync.dma_start(lcT[:], lc_featT[:])
            for i in range(2):
                nc.sync.dma_start(wd_s[i][:], wdT[i])
                # up weights: rows 0-63 AND rows 64-127 (row tiling pair)
                nc.sync.dma_start(wu_s[i][0:64, :], wuT[i])
                nc.sync.dma_start(wu_s[i][64:128, :], wuT[i])

            ones1 = pers.tile([1, 128], f16, name="ones1")
            nc.vector.memset(ones1[:], 1.0)

            a_p = [pers.tile([128, 1], f32, name=f"a_p{i}") for i in range(5)]
            b_p = [pers.tile([128, 1], f32, name=f"b_p{i}") for i in range(5)]
            c_eps = pers.tile([128, 1], f32, name="c_eps")
            nc.vector.memset(c_eps[:], EPS)
            gam_s = pers.tile([128, 5], f32, name="gam_s")
            bet_s = pers.tile([128, 5], f32, name="bet_s")
            nc.sync.dma_start(gam_s[:], gam[:].rearrange("l c -> c l"))
            nc.sync.dma_start(bet_s[:], bet[:].rearrange("l c -> c l"))

            w_row = dram.tile([P], f16, name="w_row")

            def do_allreduce(idx):
                if use_cc:
                    nc.gpsimd.collective_compute(
                        "AllReduce", ALU.add, ins=[pay_i[idx].opt()],
                        outs=[pay_o[idx].opt()], replica_groups=rg)
                else:
                    nc.sync.dma_start(pay_o[idx][:], pay_i[idx][:])
            pay_sz = [512, 128, 256, 128, 256]
            pay_i = [dram.tile([pay_sz[i]], f32, name=f"pay_i{i}") for i in range(5)]
            pay_o = [dram.tile([pay_sz[i]], f32, name=f"pay_o{i}") for i in range(5)]

            # ---------- helpers ------------------------------------------
            def stats_to_sums(ag, n, npart):
                """[npart,2] (mean,var) -> (sum, sumsq)."""
                i = stats_to_sums.i = stats_to_sums.i + 1
                sums = pers.tile([128, 2], f32, name=f"sums{i}")
                m2 = pers.tile([128, 1], f32, name=f"m2_{i}")
                nc.vector.tensor_tensor(m2[:npart], ag[:npart, 0:1], ag[:npart, 0:1], ALU.mult)
                nc.scalar.mul(sums[:npart, 0:1], ag[:npart, 0:1], float(n))
                nc.vector.tensor_tensor(sums[:npart, 1:2], ag[:npart, 1:2], m2[:npart], ALU.add)
                nc.scalar.mul(sums[:npart, 1:2], sums[:npart, 1:2], float(n))
                return sums

            stats_to_sums.i = 0

            def affine_from_sums(back, li, npart, n_total):
                """back [npart,2] global (sum,sumsq) -> a_p[li], b_p[li]."""
                mean = pers.tile([128, 1], f32, name=f"mean{li}")
                var = pers.tile([128, 1], f32, name=f"var{li}")
                m2 = pers.tile([128, 1], f32, name=f"m2g{li}")
                sig = pers.tile([128, 1], f32, name=f"sig{li}")
                nc.scalar.mul(mean[:npart], back[:npart, 0:1], 1.0 / n_total)
                nc.vector.tensor_tensor(m2[:npart], mean[:npart], mean[:npart], ALU.mult)
                nc.vector.scalar_tensor_tensor(
                    var[:npart], back[:npart, 1:2], 1.0 / n_total, m2[:npart],
                    ALU.mult, ALU.subtract)
                nc.scalar.activation(sig[:npart], var[:npart], AF.Sqrt, bias=c_eps[:npart])
                nc.vector.reciprocal(sig[:npart], sig[:npart])
                nc.vector.tensor_tensor(a_p[li][:npart], gam_s[:npart, li:li + 1],
                                        sig[:npart], ALU.mult)
                nc.vector.tensor_tensor(b_p[li][:npart], mean[:npart], a_p[li][:npart],
                                        ALU.mult)
                nc.vector.tensor_tensor(b_p[li][:npart], bet_s[:npart, li:li + 1],
                                        b_p[li][:npart], ALU.subtract)

            def pack_params(li):
                """replicate a,b [0:64] -> [64:128] for packed 64-ch layers."""
                nc.sync.dma_start(a_p[li][64:128, :], a_p[li][0:64, :])
                nc.sync.dma_start(b_p[li][64:128, :], b_p[li][0:64, :])

            def reduce_pair_and_allreduce(ag, n, idx):
                """packed [128,2] -> fold halves -> AllReduce -> affine."""
                sums = stats_to_sums(ag, n, 128)
                lo = pers.tile([64, 2], f32, name=f"lo{idx}")
                nc.sync.dma_start(lo[:], sums[64:128, :])
                nc.vector.tensor_tensor(sums[0:64, :], sums[0:64, :], lo[:], ALU.add)
                nc.sync.dma_start(pay_i[idx][0:128].rearrange("(p c) -> p c", c=2),
                                  sums[0:64, :])
                do_allreduce(idx)
                back = pers.tile([128, 2], f32, name=f"backp{idx}")
                nc.sync.dma_start(back[0:64, :],
                                  pay_o[idx][0:128].rearrange("(p c) -> p c", c=2))
                affine_from_sums(back, idx, 64, N_GLOBAL)
                pack_params(idx)

            def full_allreduce(ag, n, idx):
                sums = stats_to_sums(ag, n, 128)
                nc.sync.dma_start(pay_i[idx][0:256].rearrange("(p c) -> p c", c=2),
                                  sums[:])
                do_allreduce(idx)
                back = pers.tile([128, 2], f32, name=f"backf{idx}")
                nc.sync.dma_start(back[:],
                                  pay_o[idx][0:256].rearrange("(p c) -> p c", c=2))
                affine_from_sums(back, idx, 128, N_GLOBAL)

            # ============ phase 1: conv1 + x1 stats + xyz prep ===========
            with tc.tile_pool(name="p1", bufs=1) as p1, \
                 tc.tile_pool(name="p1e", bufs=2) as p1e, \
                 tc.tile_pool(name="ps1", bufs=4, space="PSUM") as ps1:

                # --- xyz: rel0, moments, A/Bv/Cg (points-major) ----------
                xyz = p1.tile([128, 1536], f32, name="xyz")
                nc.sync.dma_start(xyz[:], knn_xyz[:])
                lcs = p1.tile([128, 48], f32, name="lcs")
                nc.sync.dma_start(lcs[:], lc_small[:])
                rel0 = p1.tile([128, 1536], f32, name="rel0")
                lc_b = lcs[:].rearrange("p (g c) -> p g c", c=3).unsqueeze(2) \
                    .broadcast_to([128, 16, 32, 3])
                nc.vector.tensor_tensor(
                    rel0[:].rearrange("p (g k c) -> p g k c", k=32, c=3),
                    xyz[:].rearrange("p (g k c) -> p g k c", k=32, c=3),
                    lc_b, ALU.subtract)
                sq = p1.tile([128, 1536], f32, name="sq")
                nc.vector.tensor_tensor(sq[:], rel0[:], rel0[:], ALU.mult)
                A_ = p1.tile([128, 512], f32, name="A_")
                nc.vector.tensor_reduce(
                    A_[:], sq[:].rearrange("p (n c) -> p n c", c=3), AX.X, ALU.add)
                s2part = p1.tile([128, 1], f32, name="s2part")
                nc.vector.tensor_reduce(s2part[:], sq[:], AX.X, ALU.add)
                s1part = p1.tile([128, 1], f32, name="s1part")
                nc.vector.tensor_reduce(s1part[:], rel0[:], AX.X, ALU.add)
                bv_t = p1.tile([128, 1536], f32, name="bv_t", tag="sq")
                nc.vector.tensor_tensor(
                    bv_t[:].rearrange("p (g k c) -> p g k c", k=32, c=3),
                    rel0[:].rearrange("p (g k c) -> p g k c", k=32, c=3),
                    lc_b, ALU.mult)
                Bv = p1.tile([128, 512], f32, name="Bv")
                nc.vector.tensor_reduce(
                    Bv[:], bv_t[:].rearrange("p (n c) -> p n c", c=3), AX.X, ALU.add)
                lsq = p1.tile([128, 48], f32, name="lsq")
                nc.vector.tensor_tensor(lsq[:], lcs[:], lcs[:], ALU.mult)
                Cg = p1.tile([128, 16], f32, name="Cg")
                nc.vector.tensor_reduce(
                    Cg[:], lsq[:].rearrange("p (g c) -> p g c", c=3), AX.X, ALU.add)

                # --- main conv1 loop (chunked direct loads) --------------
                for ch in range(NCH):
                    est = p1e.tile([67, CH], f16, name="est")
                    nc.sync.dma_start(est[:], knn_featT[:, CH * ch:CH * (ch + 1)])
                    for s in range(CH // NP):
                        i = (CH // NP) * ch + s
                        xp = ps1.tile([128, 512], f32, name="xp")
                        nc.tensor.matmul(xp[:], w1a_s[:],
                                         est[:, NP * s:NP * (s + 1)],
                                         start=True, stop=False)
                        nc.tensor.matmul(
                            xp[:], w1b_s[:],
                            lcT[:, 16 * i:16 * (i + 1)].unsqueeze(2)
                            .broadcast_to([64, 16, 32]),
                            start=False, stop=True)
                        nc.scalar.copy(x_slot[:, NP * i:NP * (i + 1)], xp[:])
                        nc.vector.bn_stats(st[:, i, :],
                                           x_slot[:, NP * i:NP * (i + 1)])

                # --- AR1: x1 stats + rel0 moments ------------------------
                ag = p1.tile([128, 2], f32, name="ag")
                nc.vector.bn_aggr(ag[:], st[:])
                sums = stats_to_sums(ag, P, 128)
                nc.sync.dma_start(pay_i[0][0:256].rearrange("(p c) -> p c", c=2), sums[:])
                nc.sync.dma_start(pay_i[0][256:384].rearrange("(p c) -> p c", c=1), s2part[:])
                nc.sync.dma_start(pay_i[0][384:512].rearrange("(p c) -> p c", c=1), s1part[:])
                do_allreduce(0)
                back = p1.tile([128, 2], f32, name="back")
                nc.sync.dma_start(back[:], pay_o[0][0:256].rearrange("(p c) -> p c", c=2))
                affine_from_sums(back, 0, 128, N_GLOBAL)
                s2row = p1.tile([1, 128], f32, name="s2row")
                nc.sync.dma_start(s2row[:], pay_o[0][256:384].rearrange("(c n) -> c n", c=1))
                s1row = p1.tile([1, 128], f32, name="s1row")
                nc.sync.dma_start(s1row[:], pay_o[0][384:512].rearrange("(c n) -> c n", c=1))
                s2 = p1.tile([1, 1], f32, name="s2")
                nc.vector.tensor_reduce(s2[:], s2row[:], AX.X, ALU.add)
                s1 = p1.tile([1, 1], f32, name="s1")
                nc.vector.tensor_reduce(s1[:], s1row[:], AX.X, ALU.add)
                # std = sqrt((S2 - S1^2/N3)/(N3-1)) + 1e-5   (ddof=1)
                mrel = p1.tile([1, 1], f32, name="mrel")
                nc.scalar.mul(mrel[:], s1[:], 1.0 / N3)
                nc.vector.tensor_tensor(mrel[:], mrel[:], s1[:], ALU.mult)
                nc.vector.tensor_tensor(mrel[:], s2[:], mrel[:], ALU.subtract)
                stdv = p1.tile([1, 1], f32, name="stdv")
                nc.scalar.activation(stdv[:], mrel[:], AF.Sqrt, scale=1.0 / (N3 - 1))
                nc.scalar.activation(stdv[:], stdv[:], AF.Identity, bias=c_eps[0:1])
                rstd = p1.tile([1, 1], f32, name="rstd")
                nc.vector.reciprocal(rstd[:], stdv[:])
                rstd_b = p1.tile([128, 1], f32, name="rstd_b")
                nc.gpsimd.partition_broadcast(rstd_b[:], rstd[:])
                rstd2_b = p1.tile([128, 1], f32, name="rstd2_b")
                nc.vector.tensor_tensor(rstd2_b[:], rstd_b[:], rstd_b[:], ALU.mult)
                n2rstd_b = p1.tile([128, 1], f32, name="n2rstd_b")
                nc.scalar.mul(n2rstd_b[:], rstd_b[:], -2.0)

                # d2 = rstd^2*A - 2*rstd*Bv + Cg(bcast); w = exp(-sqrt(d2)/2)
                d2 = p1.tile([128, 512], f32, name="d2", tag="xyz")
                nc.vector.scalar_tensor_tensor(
                    d2[:].rearrange("p (g k) -> p g k", k=32),
                    Bv[:].rearrange("p (g k) -> p g k", k=32), n2rstd_b[:],
                    Cg[:].unsqueeze(2).broadcast_to([128, 16, 32]),
                    ALU.mult, ALU.add)
                nc.vector.scalar_tensor_tensor(
                    d2[:], A_[:], rstd2_b[:], d2[:], ALU.mult, ALU.add)
                distt = p1.tile([128, 512], f32, name="distt", tag="A_")
                nc.scalar.activation(distt[:], d2[:], AF.Sqrt)
                w_pm = p1.tile([128, 512], f16, name="w_pm")
                nc.scalar.activation(w_pm[:], distt[:], AF.Exp, scale=-0.5)
                nc.sync.dma_start(w_row[:].rearrange("(p n) -> p n", n=512), w_pm[:])

            # ============ phase 2: xw + h0 stats =========================
            with tc.tile_pool(name="p2s", bufs=4) as p2s, \
                 tc.tile_pool(name="ps2h", bufs=4, space="PSUM") as ps2h, \
                 tc.tile_pool(name="ps2w", bufs=2, space="PSUM") as ps2w:

                def make_xw(t, use_pe):
                    """x_slot tile t: x1 -> relu(a1*x1+b1)*w (in place).

                    The per-point weight w is broadcast across the 128
                    channel partitions either on gpsimd or via a
                    ones-matmul on the tensor engine, splitting the load.
                    """
                    cols = slice(NP * t, NP * (t + 1))
                    wt = p2s.tile([1, 512], f16, name="wt")
                    nc.sync.dma_start(
                        wt[:], w_row[NP * t:NP * (t + 1)].rearrange("(c n) -> c n", c=1))
                    xnr = p2s.tile([128, 512], f16, name="xnr")
                    nc.scalar.activation(xnr[:], x_slot[:, cols], AF.Relu,
                                         bias=b_p[0][:], scale=a_p[0][:])
                    if use_pe:
                        wbp = ps2w.tile([128, 512], f32, name="wbp")
                        nc.tensor.matmul(wbp[:], ones1[:], wt[:],
                                         start=True, stop=True)
                        nc.vector.tensor_tensor(x_slot[:, cols], xnr[:], wbp[:],
                                                ALU.mult)
                    else:
                        wb = p2s.tile([128, 512], f16, name="wb")
                        nc.gpsimd.partition_broadcast(wb[:], wt[:])
                        nc.vector.tensor_tensor(x_slot[:, cols], xnr[:], wb[:],
                                                ALU.mult)

                for j in range(NJ):
                    make_xw(j, use_pe=False)
                    make_xw(j + NJ, use_pe=True)
                    hp = ps2h.tile([128, 512], f32, name="hp")
                    nc.tensor.matmul(hp[0:64, :], wd_s[0][:],
                                     x_slot[:, NP * j:NP * (j + 1)],
                                     start=True, stop=True, tile_position=(0, 0))
                    nc.tensor.matmul(hp[64:128, :], wd_s[0][:],
                                     x_slot[:, NP * (j + NJ):NP * (j + NJ + 1)],
                                     start=True, stop=True, tile_position=(0, 64))
                    nc.vector.bn_stats(st[:, j, :], hp[:])

                ag2 = p2s.tile([128, 2], f32, name="ag2")
                nc.vector.bn_aggr(ag2[:], st[:, 0:NJ, :])
                reduce_pair_and_allreduce(ag2, HALF, 1)

            # ============ phase 3: h0 recompute + t + u0 stats ===========
            # The h recompute has no dependency on the preceding
            # AllReduce, so pairs 0..K3-1 are computed FIRST in program
            # order and banked in SBUF f16 — AR latency hides under them.
            with tc.tile_pool(name="p3s", bufs=3) as p3s, \
                 tc.tile_pool(name="p3r", bufs=1) as p3r, \
                 tc.tile_pool(name="ps3h", bufs=2, space="PSUM") as ps3h, \
                 tc.tile_pool(name="ps3u", bufs=2, space="PSUM") as ps3u:
                K3 = 48
                hst3 = []
                for j in range(K3):
                    hp = ps3h.tile([128, 512], f32, name="hp3")
                    nc.tensor.matmul(hp[0:64, :], wd_s[0][:],
                                     x_slot[:, NP * j:NP * (j + 1)],
                                     start=True, stop=True, tile_position=(0, 0))
                    nc.tensor.matmul(hp[64:128, :], wd_s[0][:],
                                     x_slot[:, NP * (j + NJ):NP * (j + NJ + 1)],
                                     start=True, stop=True, tile_position=(0, 64))
                    hs = p3r.tile([128, 512], f16, name=f"hs3_{j}")
                    nc.scalar.copy(hs[:], hp[:])
                    hst3.append(hs)

                def p3_body(j, hsrc):
                    tst = p3s.tile([128, 512], f16, name="tst")
                    nc.scalar.activation(tst[:], hsrc, AF.Relu,
                                         bias=b_p[1][:], scale=a_p[1][:])
                    upA = ps3u.tile([128, 512], f32, name="upA")
                    nc.tensor.matmul(upA[:], wu_s[0][0:64, :], tst[0:64, :],
                                     start=True, stop=True)
                    upB = ps3u.tile([128, 512], f32, name="upB")
                    nc.tensor.matmul(upB[:], wu_s[0][64:128, :], tst[64:128, :],
                                     start=True, stop=True)
                    nc.vector.bn_stats(st[:, 2 * j, :], upA[:])
                    nc.vector.bn_stats(st[:, 2 * j + 1, :], upB[:])

                for j in range(K3):
                    p3_body(j, hst3[j][:])
                for j in range(K3, NJ):
                    hp = ps3h.tile([128, 512], f32, name="hp3")
                    nc.tensor.matmul(hp[0:64, :], wd_s[0][:],
                                     x_slot[:, NP * j:NP * (j + 1)],
                                     start=True, stop=True, tile_position=(0, 0))
                    nc.tensor.matmul(hp[64:128, :], wd_s[0][:],
                                     x_slot[:, NP * (j + NJ):NP * (j + NJ + 1)],
                                     start=True, stop=True, tile_position=(0, 64))
                    p3_body(j, hp[:])

                ag3 = p3s.tile([128, 2], f32, name="ag3")
                nc.vector.bn_aggr(ag3[:], st[:])
                full_allreduce(ag3, P, 2)

            # ============ phase 4: r1 + h1 stats =========================
            with tc.tile_pool(name="p4s", bufs=4) as p4s, \
                 tc.tile_pool(name="ps4h", bufs=2, space="PSUM") as ps4h, \
                 tc.tile_pool(name="ps4u", bufs=2, space="PSUM") as ps4u, \
                 tc.tile_pool(name="ps4g", bufs=2, space="PSUM") as ps4g:

                def resid(up, t):
                    """x_slot tile t: xw -> relu((a2*u+b2) + xw) (in place)."""
                    cols = slice(NP * t, NP * (t + 1))
                    tmp = p4s.tile([128, 512], f32, name="tmp4")
                    nc.vector.scalar_tensor_tensor(
                        tmp[:], up[:], a_p[2][:], x_slot[:, cols],
                        ALU.mult, ALU.add)
                    nc.scalar.activation(x_slot[:, cols], tmp[:], AF.Relu,
                                         bias=b_p[2][:])

                K4 = 24
                hst4 = []
                with tc.tile_pool(name="p4r", bufs=1) as p4r:
                    for j in range(K4):
                        hp = ps4h.tile([128, 512], f32, name="hp4")
                        nc.tensor.matmul(hp[0:64, :], wd_s[0][:],
                                         x_slot[:, NP * j:NP * (j + 1)],
                                         start=True, stop=True,
                                         tile_position=(0, 0))
                        nc.tensor.matmul(hp[64:128, :], wd_s[0][:],
                                         x_slot[:, NP * (j + NJ):NP * (j + NJ + 1)],
                                         start=True, stop=True,
                                         tile_position=(0, 64))
                        hs = p4r.tile([128, 512], f16, name=f"hs4_{j}")
                        nc.scalar.copy(hs[:], hp[:])
                        hst4.append(hs)

                    def p4_body(j, hsrc):
                        tst = p4s.tile([128, 512], f16, name="tst4")
                        nc.scalar.activation(tst[:], hsrc, AF.Relu,
                                             bias=b_p[1][:], scale=a_p[1][:])
                        upA = ps4u.tile([128, 512], f32, name="upA4")
                        nc.tensor.matmul(upA[:], wu_s[0][0:64, :], tst[0:64, :],
                                         start=True, stop=True)
                        upB = ps4u.tile([128, 512], f32, name="upB4")
                        nc.tensor.matmul(upB[:], wu_s[0][64:128, :], tst[64:128, :],
                                         start=True, stop=True)
                        resid(upA, j)
                        resid(upB, j + NJ)
                        hg = ps4g.tile([128, 512], f32, name="hg4")
                        nc.tensor.matmul(hg[0:64, :], wd_s[1][:],
                                         x_slot[:, NP * j:NP * (j + 1)],
                                         start=True, stop=True,
                                         tile_position=(0, 0))
                        nc.tensor.matmul(hg[64:128, :], wd_s[1][:],
                                         x_slot[:, NP * (j + NJ):NP * (j + NJ + 1)],
                                         start=True, stop=True,
                                         tile_position=(0, 64))
                        nc.vector.bn_stats(st[:, j, :], hg[:])

                    for j in range(K4):
                        p4_body(j, hst4[j][:])
                    for j in range(K4, NJ):
                        hp = ps4h.tile([128, 512], f32, name="hp4")
                        nc.tensor.matmul(hp[0:64, :], wd_s[0][:],
                                         x_slot[:, NP * j:NP * (j + 1)],
                                         start=True, stop=True,
                                         tile_position=(0, 0))
                        nc.tensor.matmul(hp[64:128, :], wd_s[0][:],
                                         x_slot[:, NP * (j + NJ):NP * (j + NJ + 1)],
                                         start=True, stop=True,
                                         tile_position=(0, 64))
                        p4_body(j, hp[:])

                ag4 = p4s.tile([128, 2], f32, name="ag4")
                nc.vector.bn_aggr(ag4[:], st[:, 0:NJ, :])
                reduce_pair_and_allreduce(ag4, HALF, 3)

            # ============ phase 5: h1 recompute + t1 + u1 stats ==========
            with tc.tile_pool(name="p5s", bufs=3) as p5s, \
                 tc.tile_pool(name="p5r", bufs=1) as p5r, \
                 tc.tile_pool(name="ps5h", bufs=2, space="PSUM") as ps5h, \
                 tc.tile_pool(name="ps5u", bufs=2, space="PSUM") as ps5u:
                K5 = 48
                hst5 = []
                for j in range(K5):
                    hp = ps5h.tile([128, 512], f32, name="hp5")
                    nc.tensor.matmul(hp[0:64, :], wd_s[1][:],
                                     x_slot[:, NP * j:NP * (j + 1)],
                                     start=True, stop=True, tile_position=(0, 0))
                    nc.tensor.matmul(hp[64:128, :], wd_s[1][:],
                                     x_slot[:, NP * (j + NJ):NP * (j + NJ + 1)],
                                     start=True, stop=True, tile_position=(0, 64))
                    hs = p5r.tile([128, 512], f16, name=f"hs5_{j}")
                    nc.scalar.copy(hs[:], hp[:])
                    hst5.append(hs)

                def p5_body(j, hsrc):
                    tst = p5s.tile([128, 512], f16, name="tst5")
                    nc.scalar.activation(tst[:], hsrc, AF.Relu,
                                         bias=b_p[3][:], scale=a_p[3][:])
                    upA = ps5u.tile([128, 512], f32, name="upA5")
                    nc.tensor.matmul(upA[:], wu_s[1][0:64, :], tst[0:64, :],
                                     start=True, stop=True)
                    upB = ps5u.tile([128, 512], f32, name="upB5")
                    nc.tensor.matmul(upB[:], wu_s[1][64:128, :], tst[64:128, :],
                                     start=True, stop=True)
                    nc.vector.bn_stats(st[:, 2 * j, :], upA[:])
                    nc.vector.bn_stats(st[:, 2 * j + 1, :], upB[:])

                for j in range(K5):
                    p5_body(j, hst5[j][:])
                for j in range(K5, NJ):
                    hp = ps5h.tile([128, 512], f32, name="hp5")
                    nc.tensor.matmul(hp[0:64, :], wd_s[1][:],
                                     x_slot[:, NP * j:NP * (j + 1)],
                                     start=True, stop=True, tile_position=(0, 0))
                    nc.tensor.matmul(hp[64:128, :], wd_s[1][:],
                                     x_slot[:, NP * (j + NJ):NP * (j + NJ + 1)],
                                     start=True, stop=True, tile_position=(0, 64))
                    p5_body(j, hp[:])

                ag5 = p5s.tile([128, 2], f32, name="ag5")
                nc.vector.bn_aggr(ag5[:], st[:])
                full_allreduce(ag5, P, 4)

            # ============ phase 6: final =================================
            # Output is written per contiguous 4096-point chunk; pair j
            # produces tile j (first half) and tile j+NJ (second half),
            # so chunk c of each half fills from pairs 8c..8c+7.
            with tc.tile_pool(name="p6s", bufs=4) as p6s, \
                 tc.tile_pool(name="p6o", bufs=2) as p6o, \
                 tc.tile_pool(name="ps6h", bufs=2, space="PSUM") as ps6h, \
                 tc.tile_pool(name="ps6u", bufs=2, space="PSUM") as ps6u:

                def final(up, t, ost, s):
                    cols = slice(NP * t, NP * (t + 1))
                    tmp = p6s.tile([128, 512], f32, name="tmp6")
                    nc.vector.scalar_tensor_tensor(
                        tmp[:], up[:], a_p[4][:], x_slot[:, cols],
                        ALU.mult, ALU.add)
                    nc.scalar.activation(ost[:, NP * s:NP * (s + 1)], tmp[:],
                                         AF.Relu, bias=b_p[4][:])

                K6 = 16
                hst6 = []
                with tc.tile_pool(name="p6r", bufs=1) as p6r:
                    for j in range(K6):
                        hp = ps6h.tile([128, 512], f32, name="hp6")
                        nc.tensor.matmul(hp[0:64, :], wd_s[1][:],
                                         x_slot[:, NP * j:NP * (j + 1)],
                                         start=True, stop=True,
                                         tile_position=(0, 0))
                        nc.tensor.matmul(hp[64:128, :], wd_s[1][:],
                                         x_slot[:, NP * (j + NJ):NP * (j + NJ + 1)],
                                         start=True, stop=True,
                                         tile_position=(0, 64))
                        hs = p6r.tile([128, 512], f16, name=f"hs6_{j}")
                        nc.scalar.copy(hs[:], hp[:])
                        hst6.append(hs)

                    for c in range(NCH // 2):
                        ostA = p6o.tile([128, CH], f16, name="ostA")
                        ostB = p6o.tile([128, CH], f16, name="ostB")
                        for s in range(CH // NP):
                            j = (CH // NP) * c + s
                            if j < K6:
                                hsrc = hst6[j][:]
                            else:
                                hp = ps6h.tile([128, 512], f32, name="hp6")
                                nc.tensor.matmul(hp[0:64, :], wd_s[1][:],
                                                 x_slot[:, NP * j:NP * (j + 1)],
                                                 start=True, stop=True,
                                                 tile_position=(0, 0))
                                nc.tensor.matmul(
                                    hp[64:128, :], wd_s[1][:],
                                    x_slot[:, NP * (j + NJ):NP * (j + NJ + 1)],
                                    start=True, stop=True, tile_position=(0, 64))
                                hsrc = hp[:]
                            tst = p6s.tile([128, 512], f16, name="tst6")
                            nc.scalar.activation(tst[:], hsrc, AF.Relu,
                                                 bias=b_p[3][:], scale=a_p[3][:])
                            upA = ps6u.tile([128, 512], f32, name="upA6")
                            nc.tensor.matmul(upA[:], wu_s[1][0:64, :], tst[0:64, :],
                                             start=True, stop=True)
                            upB = ps6u.tile([128, 512], f32, name="upB6")
                            nc.tensor.matmul(upB[:], wu_s[1][64:128, :],
                                             tst[64:128, :], start=True, stop=True)
                            final(upA, j, ostA, s)
                            final(upB, j + NJ, ostB, s)
                        nc.sync.dma_start(out[:, CH * c:CH * (c + 1)], ostA[:])
                        nc.sync.dma_start(out[:, HALF + CH * c:HALF + CH * (c + 1)],
                                          ostB[:])

    nc.compile()
    return nc


def _prep_inputs(lc_xyz, lc_feat, knn_xyz, knn_feat, w1, bn1_g, bn1_b,
                 wd, bd, dn_g, dn_b, wu, bu, up_g, up_b):
    f16 = np.float16
    w1aT = np.ascontiguousarray(w1[:, :67].T).astype(f16)
    w1bT = np.ascontiguousarray(w1[:, 67:].T).astype(f16)
    wdT = np.ascontiguousarray(np.transpose(wd, (0, 2, 1))).astype(f16)  # [2,128,64]
    wuT = np.ascontiguousarray(np.transpose(wu, (0, 2, 1))).astype(f16)  # [2,64,128]
    gam = np.zeros((5, 128), np.float32)
    bet = np.zeros((5, 128), np.float32)
    gam[0], bet[0] = bn1_g, bn1_b
    gam[1, :64], bet[1, :64] = dn_g[0], dn_b[0]
    gam[2], bet[2] = up_g[0], up_b[0]
    gam[3, :64], bet[3, :64] = dn_g[1], dn_b[1]
    gam[4], bet[4] = up_g[1], up_b[1]
    shared = dict(w1aT=w1aT, w1bT=w1bT, wdT=wdT, wuT=wuT, gam=gam, bet=bet)
    in_maps = []
    for b in range(B):
        m = dict(shared)
        m["knn_featT"] = np.ascontiguousarray(
            knn_feat[b].reshape(P, 67).T.astype(f16))
        m["knn_xyz"] = np.ascontiguousarray(knn_xyz[b].reshape(128, 1536))
        m["lc_small"] = np.ascontiguousarray(lc_xyz[b].reshape(128, 48))
        m["lc_featT"] = np.ascontiguousarray(lc_feat[b].T.astype(f16))
        in_maps.append(m)
    return in_maps


def get_nc():
    if "nc" not in _CACHE:
        _CACHE["nc"] = _build(8)
    return _CACHE["nc"]


def make_runner(nc, n_cores=8):
    """Build the shard_map'd executable once; returns a run callable.

    Modeled on bass2jax.run_bass_via_pjrt, but caches the jitted callable
    so repeated invocations don't re-trace (needed for timing loops).
    """
    import jax
    from jax.sharding import Mesh, PartitionSpec
    from jax.experimental.shard_map import shard_map
    from concourse import bass2jax
    from concourse import mybir as _mybir

    bass2jax.install_neuronx_cc_hook()
    partition_name = nc.partition_id_tensor.name if nc.partition_id_tensor else None
    in_names, out_names, out_avals, zero_outs = [], [], [], []
    for alloc in nc.m.functions[0].allocations:
        if not isinstance(_mybir.MemoryLocationSet, type) or not isinstance(
                alloc, _mybir.MemoryLocationSet):
            continue
        name = alloc.memorylocations[0].name
        if alloc.kind == "ExternalInput":
            if name != partition_name:
                in_names.append(name)
        elif alloc.kind == "ExternalOutput":
            out_names.append(name)
            shape = tuple(alloc.tensor_shape)
            dtype = _mybir.dt.np(alloc.dtype)
            out_avals.append(jax.core.ShapedArray(shape, dtype))
            zero_outs.append(np.zeros(shape, dtype))
    n_params = len(in_names)
    all_names = in_names + out_names
    if partition_name is not None:
        all_names = all_names + [partition_name]

    def _body(*args):
        operands = list(args)
        if partition_name is not None:
            operands.append(bass2jax.partition_id_tensor())
        outs = bass2jax._bass_exec_p.bind(
            *operands,
            out_avals=tuple(out_avals),
            in_names=tuple(all_names),
            out_names=tuple(out_names),
            lowering_input_output_aliases=(),
            sim_require_finite=True,
            sim_require_nnan=True,
            nc=nc,
        )
        return tuple(outs)

    devices = jax.devices()[:n_cores]
    mesh = Mesh(np.asarray(devices), ("core",))
    n_outs = len(out_names)
    sharded = jax.jit(
        shard_map(_body, mesh=mesh,
                  in_specs=(PartitionSpec("core"),) * (n_params + n_outs),
                  out_specs=(PartitionSpec("core"),) * n_outs,
                  check_rep=False),
        donate_argnums=tuple(range(n_params, n_params + n_outs)),
        keep_unused=True)

    def run(in_maps, timing_reps=0):
        concat_in = [np.concatenate([np.asarray(in_maps[c][k])[None]
                                     for c in range(n_cores)], axis=0)
                     .reshape(n_cores * in_maps[0][k].shape[0],
                              *in_maps[0][k].shape[1:])
                     for k in in_names]
        concat_zeros = [np.zeros((n_cores * z.shape[0], *z.shape[1:]), z.dtype)
                        for z in zero_outs]
        out_arrs = sharded(*concat_in, *concat_zeros)
        jax.block_until_ready(out_arrs)
        times = []
        if timing_reps:
            import time
            ins_dev = jax.device_put(concat_in)
            jax.block_until_ready(ins_dev)
            for _ in range(timing_reps):
                zer_dev = jax.device_put(concat_zeros)
                jax.block_until_ready(zer_dev)
                t0 = time.perf_counter()
                o = sharded(*ins_dev, *zer_dev)
                jax.block_until_ready(o)
                times.append(time.perf_counter() - t0)
        return ({name: np.asarray(out_arrs[i]).reshape(n_cores, *out_avals[i].shape)
                 for i, name in enumerate(out_names)}, times)

    return run


def kernel(**inputs):
    inputs = {k: np.asarray(v) for k, v in inputs.items()}
    nc = get_nc()
    in_maps = _prep_inputs(**inputs)
    res = bass_utils.run_bass_kernel_spmd(nc, in_maps, core_ids=list(range(8)))
    outs = [res.results[c]["out"].astype(np.float32).reshape(128, G, KNN)
            for c in range(B)]
    return np.stack(outs, axis=0)


if __name__ == "__main__":
    import reference
    import jax.numpy as jnp
    inp = {k: np.asarray(v) for k, v in reference.setup_inputs().items()}
    got = kernel(**inp)
    exp = np.asarray(reference.reference(**{k: jnp.asarray(v) for k, v in inp.items()}))
    rel = np.linalg.norm(got - exp) / np.linalg.norm(exp)
    print("Relative error:", rel, "absmax:", np.abs(got - exp).max())
